# revision 1
# baseline (speedup 1.0000x reference)
"""Multi-head attention on 8 Trainium2 NeuronCores (head-parallel).

Problem: Q,K,V [4096,512] fp32; Wq/Wk/Wv [8,512,64]; Wo [512,512].
  out = concat_h(softmax(QWq_h (KWk_h)^T / sqrt(64)) VWv_h) @ Wo

Sharding: one head per core. Each core computes its head end-to-end plus
its slice of the output projection (out_h @ Wo[64h:64h+64, :]); the host
sums the 8 partial [4096,512] outputs.

Per-core pipeline (n = 4096 queries, m = 4096 keys, d = 64):
  P1  projections (fp32 matmul). q and k are split hi/lo into bf16 pairs
      (q = q_hi + q_lo exactly in fp32; each part bf16) so the score
      matmuls can run at bf16 rate with ~fp32 accuracy via
      s = k_hi q_hi + k_lo q_hi + k_hi q_lo (the dropped lo*lo term is
      ~1e-3 absolute on logits of O(700)). v is evicted to bf16 tiles
      [128, d+1] with a constant ones column: the ones column makes the
      attn.V matmul also produce the softmax denominator.
      1/sqrt(d) is folded into Wq on the host.
  P2  per 512-query chunk, software-pipelined one chunk ahead:
      stats pass (bf16, 2-way row-packed): natural-layout scores
        [n-tile, m] -> per-row max (DVE reduce over PSUM); row maxes are
        DMA-scattered into row 64 of the q_hi operand.
      main pass (bf16 hi/lo): transposed scores; the hi*hi matmul carries
        a 65th contraction row (k side = -1, q side = rowmax) so the PSUM
        result is qk^T - rowmax directly; the two cross terms are K=64
        and run 2-way row-packed across m-tiles. ACT exp evicts
        PSUM -> SBUF bf16 attn^T.
      attn.V (bf16): accumulate outT [d+1, 512] in PSUM over all 32
        m-tiles; row d is the softmax sum.
  P3  Wo (bf16): partial[n-tile,512] = outT^T @ wo, scaled by 1/sum per
      query row (DVE per-partition scalar) on PSUM->SBUF eviction.

The double scores computation exists because softmax needs the query
index on partitions (per-partition reduce) while the attn.V matmul needs
the key index on partitions; scores are computed in both layouts (the
stats one only feeds the max, so it can be sloppy) rather than
transposing a 64MB attn matrix on-chip.

Row maxes ride in bf16: softmax is shift-invariant, so subtracting a
max that is off by <3 only scales exp values by <e^3, which the
self-consistent denominator (computed from the same bf16 attn weights)
cancels exactly.
"""

from contextlib import ExitStack

import numpy as np

N = 4096
DIM = 512
H = 8
D = 64
P = 128
CH = 512  # query columns per era (chunk)


def build_head_kernel(ctx, tc, outs, ins, n=N, dim=DIM, d=D):
    import concourse.bass as bass
    import concourse.mybir as mybir
    from concourse.bass import ts, ds

    nc = tc.nc
    f32 = mybir.dt.float32
    bf16 = mybir.dt.bfloat16
    AF = mybir.ActivationFunctionType

    KC = dim // P      # projection contraction chunks (4)
    NT = n // P        # 128-row tiles of n (= m tiles) (32)
    NCH = n // CH      # eras (8)
    NTC = CH // P      # n-tiles per era (4)
    MC = n // 512      # 512-wide m-chunks for the stats pass (8)
    PAIRS = MC // 2    # packed stats pairs per n-tile (4)
    GRP = NT // 2      # main groups per era, 2 m-tiles each (16)
    assert n % 1024 == 0 and dim % P == 0 and CH == 512

    qth_d, qtl_d = ins["QTH"], ins["QTL"]
    kth_d, ktl_d = ins["KTH"], ins["KTL"]
    vt_d = ins["VT"]
    wqh_d, wql_d = ins["wqh"], ins["wql"]
    wkh_d, wkl_d = ins["wkh"], ins["wkl"]
    wv_d, wo_d = ins["wv"], ins["wo"]
    out_d = outs["out"]

    singles = ctx.enter_context(tc.tile_pool(name="singles", bufs=1))

    # Persistent SBUF tensors.  *dup tiles hold the same data relocated to
    # partitions 64..127 so pairs of K<=64 matmuls can run concurrently in
    # distinct PE row-groups (tile_position row packing).
    Ah_ev = singles.tile([d + 1, n], bf16)  # q_hi; row d = rowmax (even eras)
    Ah_od = singles.tile([d + 1, n], bf16)  # q_hi; row d = rowmax (odd eras)
    Al = singles.tile([d, n], bf16)         # q_lo
    Bh = singles.tile([d + 1, n], bf16)     # k_hi; row d = -1
    Bl = singles.tile([d, n], bf16)         # k_lo
    qdup = singles.tile([P, n], bf16)       # rows 64..127 = q_hi
    kdup = singles.tile([P, n], bf16)       # rows 64..127 = k_hi
    qldup = singles.tile([P, n], bf16)      # rows 64..127 = q_lo
    kldup = singles.tile([P, n], bf16)      # rows 64..127 = k_lo
    v_sb = singles.tile([P, NT, d + 1], bf16)  # v tiles + ones column
    outT = singles.tile([d, n], bf16)       # attn_u @ v
    sumx = singles.tile([1, n], f32)        # softmax denominators
    rsum = singles.tile([P, NT], f32)       # sumexp gathered per n-tile
    rinv = singles.tile([P, NT], f32)
    wqh_sb = singles.tile([P, KC, d], bf16)
    wql_sb = singles.tile([P, KC, d], bf16)
    wkh_sb = singles.tile([P, KC, d], bf16)
    wkl_sb = singles.tile([P, KC, d], bf16)
    wv_sb = singles.tile([P, KC, d], f32)
    wo_sb = singles.tile([d, dim], f32)
    wo_bf = singles.tile([d, dim], bf16)

    def _load_w(w_sb, w_d):
        nc.sync.dma_start(out=w_sb, in_=w_d.rearrange("(c p) e -> p c e", p=P))

    # stats machinery: PSUM pool opens before P1 so chunk-0 stats can run
    # inside the (DMA-bound) projection phase as its k-chunks land
    st_pool = ctx.enter_context(tc.tile_pool(name="st_ps_pool", bufs=1, space="PSUM"))
    nmax_pool = ctx.enter_context(tc.tile_pool(name="nmax_pool", bufs=5))

    # stats nmax tiles are per-n-tile scratch; chunk-0 emission is p-major
    # (pair index advances as k-projection chunks complete), so all NTC nmax
    # tiles are live at once -- nmax_pool bufs covers NTC + the cmax tile
    nmax_tiles = {}

    def stats_item(c, g):
        """One 2-way row-packed pair of natural-layout score matmuls."""
        j, p = divmod(g, PAIRS)
        gt = c * NTC + j  # global n-tile
        if p == 0:
            nmax_tiles[j] = nmax_pool.tile([P, PAIRS], bf16, tag="nmax",
                                           name="nmax")
        st_ps = st_pool.tile([P, 1024], f32)
        nc.tensor.matmul(st_ps[:, 0:512], lhsT=Ah_ev[0:d, ts(gt, P)],
                         rhs=Bh[0:d, ts(2 * p, 512)], start=True, stop=True)
        nc.tensor.matmul(st_ps[:, 512:1024], lhsT=qdup[d:2 * d, ts(gt, P)],
                         rhs=kdup[d:2 * d, ts(2 * p + 1, 512)], start=True, stop=True)
        nc.vector.reduce_max(nmax_tiles[j][:, p:p + 1], st_ps,
                             axis=mybir.AxisListType.X)
        if p == PAIRS - 1:
            if j == 0:  # first finished n-tile of this chunk: alloc gather buf
                stats_item.cmax = nmax_pool.tile([P, NTC], bf16, tag="cmax")
            nc.vector.reduce_max(stats_item.cmax[:, j:j + 1], nmax_tiles[j],
                                 axis=mybir.AxisListType.X)
        if g == NTC * PAIRS - 1:
            At = Ah_ev if c % 2 == 0 else Ah_od
            # scatter per-row maxes into row d: column n = c*CH + jj*P + row
            for jj in range(NTC):
                nc.sync.dma_start(out=At[d:d + 1, ds(c * CH + jj * P, P)],
                                  in_=stats_item.cmax[:, jj:jj + 1])

    # ---- P1: projections (bf16 hi/lo), chunk-0 stats folded in ----
    NB = n // 512
    pending = []  # chunk-0 stats thunks, emitted at spaced slots for overlap

    def flush_one():
        if pending:
            pending.pop(0)()

    with tc.tile_pool(name="pstream", bufs=3) as pstream, \
         tc.tile_pool(name="pq_ps", bufs=2, space="PSUM") as pq_pool, \
         tc.tile_pool(name="pk_ps", bufs=2, space="PSUM") as pk_pool, \
         tc.tile_pool(name="pv_ps", bufs=2, space="PSUM") as pv_pool:

        def load_stream(t_d, tag, dtype, cols, nbs):
            t = pstream.tile([P, KC, cols], dtype, tag=tag, name=tag)
            nc.sync.dma_start(out=t, in_=t_d[:, nbs].rearrange("(c p) x -> p c x", p=P))
            return [t[:, kc, :] for kc in range(KC)]

        def v_tile(mt):
            vt_t = load_stream(vt_d, "vt", f32, P, ts(mt, P))
            ps_v = pv_pool.tile([P, d], f32)
            for kc in range(KC):
                nc.tensor.matmul(ps_v, lhsT=vt_t[kc], rhs=wv_sb[:, kc, :],
                                 start=(kc == 0), stop=(kc == KC - 1))
            nc.vector.tensor_copy(v_sb[:, mt, 0:d], ps_v)

        for nb in range(NB):
            nbs = ds(nb * 512, 512)
            qth_t = load_stream(qth_d, "qth", bf16, 512, nbs)
            if nb == 0:
                # weights are DMA-queued in first-use order behind the data
                # they pair with, so the first matmul waits on the minimum
                # number of queued bytes
                _load_w(wqh_sb, wqh_d)
                _load_w(wql_sb, wql_d)
            qtl_t = load_stream(qtl_d, "qtl", bf16, 512, nbs)
            if nb == 0:
                _load_w(wkh_sb, wkh_d)
                _load_w(wkl_sb, wkl_d)
            kth_t = load_stream(kth_d, "kth", bf16, 512, nbs)
            ktl_t = load_stream(ktl_d, "ktl", bf16, 512, nbs)
            if nb == 0:
                _load_w(wv_sb, wv_d)
                nc.sync.dma_start(out=wo_sb, in_=wo_d)
                nc.vector.tensor_copy(wo_bf, wo_sb)
                nc.vector.memset(Bh[d:d + 1, :], -1.0)
                nc.vector.memset(v_sb[:, :, d:d + 1], 1.0)
            ps_q = pq_pool.tile([d, 512], f32)
            ps_k = pk_pool.tile([d, 512], f32)
            # q = Wq^T Q via bf16 hi/lo (lo*lo dropped)
            terms_q = [(wqh_sb, qth_t), (wqh_sb, qtl_t), (wql_sb, qth_t)]
            for i, (w, x) in enumerate(terms_q):
                for kc in range(KC):
                    nc.tensor.matmul(ps_q, lhsT=w[:, kc, :], rhs=x[kc],
                                     start=(i == 0 and kc == 0),
                                     stop=(i == 2 and kc == KC - 1))
            nc.scalar.copy(Ah_ev[0:d, nbs], ps_q)                  # hi = bf16(q)
            nc.vector.tensor_sub(Al[:, nbs], ps_q, Ah_ev[0:d, nbs])  # lo = q - hi
            flush_one()
            terms_k = [(wkh_sb, kth_t), (wkh_sb, ktl_t), (wkl_sb, kth_t)]
            for i, (w, x) in enumerate(terms_k):
                for kc in range(KC):
                    nc.tensor.matmul(ps_k, lhsT=w[:, kc, :], rhs=x[kc],
                                     start=(i == 0 and kc == 0),
                                     stop=(i == 2 and kc == KC - 1))
            nc.scalar.copy(Bh[0:d, nbs], ps_k)
            nc.vector.tensor_sub(Bl[:, nbs], ps_k, Bh[0:d, nbs])
            flush_one()
            # relocate this chunk's hi/lo copies to partitions 64..127
            # (SBUF->SBUF DMA can cross partitions; compute engines cannot)
            nc.sync.dma_start(out=qdup[d:2 * d, nbs], in_=Ah_ev[0:d, nbs])
            nc.sync.dma_start(out=kdup[d:2 * d, nbs], in_=Bh[0:d, nbs])
            nc.sync.dma_start(out=qldup[d:2 * d, nbs], in_=Al[:, nbs])
            nc.sync.dma_start(out=kldup[d:2 * d, nbs], in_=Bl[:, nbs])
            flush_one()
            v_tile(2 * nb)
            flush_one()
            v_tile(2 * nb + 1)
            flush_one()
            if nb % 2 == 1:
                # k-chunks 2p, 2p+1 (p = nb//2) are now projected+relocated:
                # queue the chunk-0 stats pairs that contract against them
                p = nb // 2
                for j in range(NTC):
                    pending.append(lambda j=j, p=p: stats_item(0, j * PAIRS + p))
        for mt in range(2 * NB, NT):
            v_tile(mt)
            flush_one()
        while pending:
            flush_one()
        # odd-era copy of q_hi (separate tile so era c+1's rowmax scatter
        # never WARs era c's score matmul reads)
        nc.sync.dma_start(out=Ah_od[0:d, :], in_=Ah_ev[0:d, :])

    # ---- P2: stats (chunk c+1) interleaved with main (chunk c) ----
    with tc.tile_pool(name="sc_ps_pool", bufs=2, space="PSUM") as sc_pool, \
         tc.tile_pool(name="av_ps_pool", bufs=2, space="PSUM") as av_pool, \
         tc.tile_pool(name="att_pool", bufs=5) as att_pool:

        def wo_tile(t, ps_tile, o_sb):
            """Output-projection for n-tile t, scaled by 1/sumexp on eviction."""
            nc.tensor.matmul(ps_tile, lhsT=outT[:, ts(t, P)], rhs=wo_bf,
                             start=True, stop=True)
            # alternate eviction engine so neither DVE nor ACT serializes
            if t % 2 == 0:
                nc.vector.tensor_scalar_mul(o_sb, ps_tile, rinv[:, t:t + 1])
            else:
                nc.scalar.mul(o_sb, ps_tile, rinv[:, t:t + 1])
            nc.sync.dma_start(out=out_d[ts(t, P), :], in_=o_sb)

        def era(c):
            """Main pass for chunk c; stats for chunk c+1 interleaved."""
            At = Ah_ev if c % 2 == 0 else Ah_od
            cs = ds(c * CH, CH)
            r_hi65 = At[:, cs]        # [d+1, 512], row d = rowmax
            r_hi = At[0:d, cs]
            r_lo = Al[:, cs]
            r_hi_b = qdup[d:2 * d, cs]
            r_lo_b = qldup[d:2 * d, cs]
            av_ps = av_pool.tile([d + 1, 512], f32, tag="av")
            att_fifo = []  # (att_tile, g) awaiting attn.V, deferred 2 groups

            def emit_av(att_t, g):
                nc.tensor.matmul(av_ps, lhsT=v_sb[:, 2 * g, :], rhs=att_t[:, 0:512],
                                 start=(g == 0), stop=False)
                nc.tensor.matmul(av_ps, lhsT=v_sb[:, 2 * g + 1, :], rhs=att_t[:, 512:1024],
                                 start=False, stop=(g == GRP - 1))

            for g in range(GRP):
                # 2 stats pairs per group over the first half of the era, so
                # the rowmax scatter completes with half an era of slack
                # before era c+1's first score matmul reads it
                if c + 1 < NCH:
                    for k in (2 * g, 2 * g + 1):
                        if k < NTC * PAIRS:
                            stats_item(c + 1, k)
                mta, mtb = ts(2 * g, P), ts(2 * g + 1, P)
                sc_ps = sc_pool.tile([P, 1024], f32, tag="sc")
                att_t = att_pool.tile([P, 1024], bf16, tag="att")
                sa, sb = sc_ps[:, 0:512], sc_ps[:, 512:1024]
                # hi*hi with the rowmax-subtraction row (K=65, unpackable)
                nc.tensor.matmul(sa, lhsT=Bh[:, mta], rhs=r_hi65, start=True, stop=False)
                nc.tensor.matmul(sb, lhsT=Bh[:, mtb], rhs=r_hi65, start=True, stop=False)
                # cross terms, 2-way row-packed (rows 0..63 / 64..127)
                nc.tensor.matmul(sa, lhsT=Bl[:, mta], rhs=r_hi, start=False, stop=False)
                nc.tensor.matmul(sb, lhsT=kldup[d:2 * d, mtb], rhs=r_hi_b,
                                 start=False, stop=False)
                nc.tensor.matmul(sa, lhsT=Bh[0:d, mta], rhs=r_lo, start=False, stop=True)
                nc.tensor.matmul(sb, lhsT=kdup[d:2 * d, mtb], rhs=r_lo_b,
                                 start=False, stop=True)
                nc.scalar.activation(att_t, sc_ps, AF.Exp)
                # defer attn.V two groups so the exp it reads is long done
                # even when row-packing makes PE outpace ACT on real hardware
                att_fifo.append((att_t, g))
                if len(att_fifo) > 2:
                    emit_av(*att_fifo.pop(0))
                if c == NCH - 1 and 2 * g < NT - NTC:
                    # the stats PSUM banks are idle in the last era (there is
                    # no chunk-NCH stats pass): run the earlier chunks' output
                    # projection there, overlapped with this era's compute
                    wops = st_pool.tile([P, 1024], f32, tag="st_ps", name="wops")
                    for i in range(2):
                        o_sb = att_pool.tile([P, dim], f32, tag="o_early",
                                             name="o_early")
                        wo_tile(2 * g + i, wops[:, i * 512:(i + 1) * 512], o_sb)
            for item in att_fifo:
                emit_av(*item)
            # evict attn_u @ v (bf16) and the sumexp row (fp32), then gather
            # the per-n-tile denominators
            nc.scalar.copy(outT[:, cs], av_ps[0:d, :])
            nc.scalar.copy(sumx[:, cs], av_ps[d:d + 1, :])
            for jj in range(NTC):
                nc.sync.dma_start(out=rsum[:, c * NTC + jj:c * NTC + jj + 1],
                                  in_=sumx[:, ds(c * CH + jj * P, P)])
            nc.vector.reciprocal(rinv[:, ds(c * NTC, NTC)], rsum[:, ds(c * NTC, NTC)])

        for c in range(NCH):
            era(c)

        # ---- P3 tail: last chunk's output projection (the rest ran in the
        # final era on the idle stats banks) ----
        for t in range(NT - NTC, NT):
            wops = st_pool.tile([P, 1024], f32, tag="st_ps", name="wops")
            o_sb = att_pool.tile([P, dim], f32, tag="o_early", name="o_early")
            wo_tile(t, wops[:, 0:512], o_sb)


def _hilo(x):
    """Split fp32 array into bf16 (hi, lo) with x ~= hi + lo."""
    import ml_dtypes

    hi = x.astype(ml_dtypes.bfloat16)
    lo = (x - hi.astype(np.float32)).astype(ml_dtypes.bfloat16)
    return np.ascontiguousarray(hi), np.ascontiguousarray(lo)


def make_in_maps(Q, K, V, Wq, Wk, Wv, Wo):
    """Host-side sharding: transpose activations, slice weights per head."""
    scale = 1.0 / np.sqrt(Wq.shape[-1])
    QTH, QTL = _hilo(np.ascontiguousarray(Q.T.astype(np.float32)))
    KTH, KTL = _hilo(np.ascontiguousarray(K.T.astype(np.float32)))
    VT = np.ascontiguousarray(V.T.astype(np.float32))
    d = Wq.shape[-1]
    in_maps = []
    for h in range(Wq.shape[0]):
        wqh, wql = _hilo(Wq[h].astype(np.float32) * scale)
        wkh, wkl = _hilo(Wk[h].astype(np.float32))
        in_maps.append({
            "QTH": QTH, "QTL": QTL, "KTH": KTH, "KTL": KTL, "VT": VT,
            "wqh": wqh, "wql": wql, "wkh": wkh, "wkl": wkl,
            "wv": np.ascontiguousarray(Wv[h].astype(np.float32)),
            "wo": np.ascontiguousarray(Wo[h * d:(h + 1) * d, :].astype(np.float32)),
        })
    return in_maps


_CACHE = {}


def _build_and_compile(n=N, dim=DIM, d=D, num_cores=H, repeats=1):
    import concourse.bass as bass
    import concourse.mybir as mybir
    import concourse.tile as tile
    from concourse import bacc

    key = (n, dim, d, num_cores, repeats)
    if key in _CACHE:
        return _CACHE[key]
    nc = bacc.Bacc("TRN2", target_bir_lowering=False, debug=False,
                   num_devices=num_cores)
    f32 = mybir.dt.float32
    bf16 = mybir.dt.bfloat16
    ins = {}
    for name in ("QTH", "QTL", "KTH", "KTL"):
        ins[name] = nc.dram_tensor(name, [dim, n], bf16, kind="ExternalInput").ap()
    ins["VT"] = nc.dram_tensor("VT", [dim, n], f32, kind="ExternalInput").ap()
    for name in ("wqh", "wql", "wkh", "wkl"):
        ins[name] = nc.dram_tensor(name, [dim, d], bf16, kind="ExternalInput").ap()
    ins["wv"] = nc.dram_tensor("wv", [dim, d], f32, kind="ExternalInput").ap()
    ins["wo"] = nc.dram_tensor("wo", [d, dim], f32, kind="ExternalInput").ap()
    outs = {"out": nc.dram_tensor("out", [n, dim], f32, kind="ExternalOutput").ap()}
    with tile.TileContext(nc) as tc:
        for _rep in range(repeats):
            with ExitStack() as ctx:
                build_head_kernel(ctx, tc, outs, ins, n=n, dim=dim, d=d)
    nc.compile()
    _CACHE[key] = nc
    return nc


def run_on_hw(in_maps, trace=False, **kwargs):
    from concourse.bass_utils import run_bass_kernel_spmd

    nc = _build_and_compile(num_cores=len(in_maps))
    return run_bass_kernel_spmd(nc, in_maps, core_ids=list(range(len(in_maps))),
                                trace=trace, **kwargs)


def kernel(Q, K, V, Wq, Wk, Wv, Wo):
    in_maps = make_in_maps(np.asarray(Q), np.asarray(K), np.asarray(V),
                           np.asarray(Wq), np.asarray(Wk), np.asarray(Wv),
                           np.asarray(Wo))
    res = run_on_hw(in_maps)
    out = np.zeros((N, DIM), dtype=np.float64)
    for r in res.results:
        out += r["out"].astype(np.float64)
    return out.astype(np.float32)


if __name__ == "__main__":
    rng = np.random.default_rng(0)
    inputs = {
        "Q": rng.standard_normal((N, DIM), dtype=np.float32),
        "K": rng.standard_normal((N, DIM), dtype=np.float32),
        "V": rng.standard_normal((N, DIM), dtype=np.float32),
        "Wq": rng.random((H, DIM, D), dtype=np.float32),
        "Wk": rng.random((H, DIM, D), dtype=np.float32),
        "Wv": rng.random((H, DIM, D), dtype=np.float32),
        "Wo": rng.random((DIM, DIM), dtype=np.float32),
    }
    out = kernel(**inputs)
    print(out.shape, out.dtype, np.abs(out).max())



# revision 16
# speedup vs baseline: 1.2096x; 1.2096x over previous
"""Multi-head attention on 8 Trainium2 NeuronCores (head-parallel).

Problem: Q,K,V [4096,512] fp32; Wq/Wk/Wv [8,512,64]; Wo [512,512].
  out = concat_h(softmax(QWq_h (KWk_h)^T / sqrt(64)) VWv_h) @ Wo

Sharding: one head per core. Each core computes its head end-to-end plus
its slice of the output projection (out_h @ Wo[64h:64h+64, :]); the host
sums the 8 partial [4096,512] outputs.

Per-core pipeline (n = 4096 queries, m = 4096 keys, d = 64):
  P1  projections. q and k run in fp32r (full-rate 4-byte matmul when the
      moving free size is >= 256; ~2^-12 relative precision) straight from
      fp32r DRAM operands -- no hi/lo splitting. v runs in fp16. q/k PSUM
      results are evicted twice: once to fp32r [65, n] operand tiles for
      the main pass, once to fp8 e4m3 hi/lo planes for the stats pass.
  P2  per 512-query chunk:
      stats pass (fp8 DoubleRow, 0.5 cycles/row): natural-layout scores
        q8 x k8 with the (hi, lo) planes as the two DoubleRow contraction
        tiles, so (q_hi + q_lo) . k_hi lands in one instruction. Per-row
        max via DVE reduce over PSUM; row maxes are DMA-scattered into
        row 64 of the fp32r q operand.
      main pass (fp32r): transposed scores; the matmul carries a 65th
        contraction row (k side = -1, q side = rowmax) so PSUM holds
        qk^T - rowmax directly. ACT exp evicts PSUM -> fp16 attn^T.
      attn.V (fp16): accumulate outT [65, 512] in PSUM over all 32
        m-tiles; row 64 (ones column of v) is the softmax denominator.
  P3  Wo (fp16): partial[n-tile,512] = outT^T @ wo scaled by 1/sum per
      query row on eviction. One tile per 4 groups runs inside the next
      era so the output DMA is spread instead of tail-heavy.

The double scores computation exists because softmax needs the query
index on partitions (per-partition reduce) while the attn.V matmul needs
the key index on partitions; the stats pass only feeds the max, so fp8
(+-2 logits on the max) is plenty: softmax is shift-invariant and the
denominator comes from the same shifted weights, so the shift error
cancels exactly. fp16 attn tiles are safe because the max error is
bounded well below the fp16 overflow point (e^11).
"""

from contextlib import ExitStack

import numpy as np

N = 4096
DIM = 512
H = 8
D = 64
P = 128
CH = 512  # query columns per era (chunk)


def build_head_kernel(ctx, tc, outs, ins, n=N, dim=DIM, d=D):
    import concourse.bass as bass
    import concourse.mybir as mybir
    from concourse.bass import ts, ds

    nc = tc.nc
    f32 = mybir.dt.float32
    f32r = mybir.dt.float32r
    f16 = mybir.dt.float16
    bf16 = mybir.dt.bfloat16
    AF = mybir.ActivationFunctionType
    X = mybir.AxisListType.X

    KC = dim // P      # projection contraction chunks (4)
    NT = n // P        # 128-row tiles of n (= m tiles) (32)
    NCH = n // CH      # eras (8)
    NTC = CH // P      # n-tiles per era (4)
    NB = n // 512      # projection column blocks (8)
    GRP = NT // 2      # main groups per era, 2 m-tiles each (16)
    assert CH == 512 and n % 1024 == 0

    qth_d, qtl_d = ins["QTH"], ins["QTL"]
    kth_d, ktl_d = ins["KTH"], ins["KTL"]
    vt_d = ins["VT"]
    wqh_d, wql_d = ins["wqh"], ins["wql"]
    wkh_d, wkl_d = ins["wkh"], ins["wkl"]
    wv_d, wo_d = ins["wv"], ins["wo"]
    out_d = outs["out"]
    # q/k projections run as fp16 hi/lo 3-term (wh.xh + wh.xl + wl.xh):
    # the scores' rank-1 outlier component (|s| ~ 1e4) demands ~2^-17
    # relative projection accuracy, which single-pass fp32r cannot give.

    singles = ctx.enter_context(tc.tile_pool(name="singles", bufs=1))

    # Persistent SBUF tensors.  QR_* row 64 carries the per-query rowmax
    # (alternating tiles so era c+1's scatter never aliases era c's reads);
    # KR row 64 is the constant -1 that turns the 65th contraction row into
    # the rowmax subtraction.
    # fp16 hi/lo score operands. Scores reach |s| ~ 1e4 (the all-positive
    # projection weights create a large shared rank-1 component) while the
    # 2e-2 gate needs score errors ~0.1, i.e. ~2^-17 relative: a single
    # fp32r pass (~2^-13) is not enough. hi*hi (K=65, with the rowmax row)
    # plus ONE fused cross pass [kl;kh] x [qh;ql] (K=128) leaves only the
    # dropped lo*lo term ~|s|*2^-22. The fp16-hi tiles double as the stats
    # operands (max error ~|s|*2^-11 cancels via the shared denominator).
    QH_ev = singles.tile([d + 1, n], f16)   # rows 0-63 q_hi; row 64 rowmax
    QH_od = singles.tile([d + 1, n], f16)
    KH = singles.tile([d + 1, n], f16)      # rows 0-63 k_hi; row 64 = -1
    QX = singles.tile([P, n], f16)          # rows 0-63 q_hi, 64-127 q_lo
    KX = singles.tile([P, n], f16)          # rows 0-63 k_lo, 64-127 k_hi
    v_sb = singles.tile([P, NT, d + 1], bf16)  # v tiles + ones column
    outT = singles.tile([d, n], bf16)       # attn_u @ v
    sumx = singles.tile([1, n], f32)        # softmax denominators
    rsum = singles.tile([P, NT], f32)       # sumexp gathered per n-tile
    rinv = singles.tile([P, NT], f32)
    wqh_sb = singles.tile([P, KC, d], f16)
    wql_sb = singles.tile([P, KC, d], f16)
    wkh_sb = singles.tile([P, KC, d], f16)
    wkl_sb = singles.tile([P, KC, d], f16)
    wv_sb = singles.tile([P, KC, d], f16)
    wo_sb = singles.tile([d, dim], f16)

    def _load_w(w_sb, w_d):
        nc.sync.dma_start(out=w_sb, in_=w_d.rearrange("(c p) e -> p c e", p=P))

    nmax_pool = ctx.enter_context(tc.tile_pool(name="nmax_pool", bufs=6))

    nmax_tiles = {}

    def stats_item(c, k, pool, tag):
        """Stats for chunk c, item k: n-tile j = k//4 vs key block i = k%4."""
        j, i = divmod(k, NTC)
        gt = c * NTC + j  # global n-tile
        if i == 0:
            nmax_tiles[j] = nmax_pool.tile([P, NTC], f32, tag="nmax",
                                           name="nmax")
        st_ps = pool.tile([P, 1024], f32, tag=tag, name="st_ps")
        nc.tensor.matmul(st_ps[:, 0:512], lhsT=QH_ev[0:d, ts(gt, P)],
                         rhs=KH[0:d, ds(i * 1024, 512)], start=True, stop=True)
        nc.tensor.matmul(st_ps[:, 512:1024], lhsT=QH_ev[0:d, ts(gt, P)],
                         rhs=KH[0:d, ds(i * 1024 + 512, 512)], start=True, stop=True)
        nc.vector.reduce_max(nmax_tiles[j][:, i:i + 1], st_ps, axis=X)
        if i == NTC - 1:
            cm = nmax_pool.tile([P, 1], f16, tag="cm", name="cm")
            nc.vector.reduce_max(cm, nmax_tiles[j], axis=X)
            At = QH_ev if c % 2 == 0 else QH_od
            # scatter per-row maxes into row 64: column n = c*CH + j*P + row
            nc.sync.dma_start(out=At[d:d + 1, ds(c * CH + j * P, P)], in_=cm)

    # ---- P1: projections (fp16 -> fp32r/fp8), chunk-0 stats folded in ----
    with tc.tile_pool(name="pstream", bufs=3) as pstream, \
         tc.tile_pool(name="ktstream", bufs=4) as ktstream, \
         tc.tile_pool(name="pq_ps", bufs=1, space="PSUM") as pq_pool, \
         tc.tile_pool(name="pk_ps", bufs=1, space="PSUM") as pk_pool, \
         tc.tile_pool(name="pv_ps", bufs=2, space="PSUM") as pv_pool, \
         tc.tile_pool(name="st0_ps", bufs=2, space="PSUM") as st0_pool:

        # weights on the ACT hwdge queue: issues in parallel with the
        # kt stream on SP, so kt0 is the first SP transfer
        def _load_w(w_sb, w_d):
            nc.scalar.dma_start(out=w_sb, in_=w_d.rearrange("(c p) e -> p c e", p=P))
        _load_w(wkh_sb, wkh_d)
        _load_w(wkl_sb, wkl_d)
        _load_w(wqh_sb, wqh_d)
        _load_w(wql_sb, wql_d)
        _load_w(wv_sb, wv_d)
        nc.scalar.dma_start(out=wo_sb, in_=wo_d)
        nc.vector.memset(KH[d:d + 1, :], -1.0)
        nc.vector.memset(v_sb[:, :, d:d + 1], 1.0)

        def q_proj(nb):
            nbs = ds(nb * 512, 512)
            qth_t = pstream.tile([P, KC, 512], f16, tag="qth", name="qth")
            nc.sync.dma_start(out=qth_t,
                              in_=qth_d[:, nbs].rearrange("(c p) x -> p c x", p=P))
            qtl_t = pstream.tile([P, KC, 512], f16, tag="qtl", name="qtl")
            nc.sync.dma_start(out=qtl_t,
                              in_=qtl_d[:, nbs].rearrange("(c p) x -> p c x", p=P))
            ps_q = pq_pool.tile([d, 512], f32)
            terms = [(wqh_sb, qth_t), (wqh_sb, qtl_t), (wql_sb, qth_t)]
            for i, (w, x) in enumerate(terms):
                for kc in range(KC):
                    nc.tensor.matmul(ps_q, lhsT=w[:, kc, :], rhs=x[:, kc, :],
                                     start=(i == 0 and kc == 0),
                                     stop=(i == 2 and kc == KC - 1))
            nc.scalar.copy(QH_ev[0:d, nbs], ps_q)
            nc.scalar.copy(QX[0:d, nbs], ps_q)
            qlt = pstream.tile([d, 512], f16, tag="qlt", name="qlt")
            nc.vector.tensor_sub(qlt, ps_q, QX[0:d, nbs])
            nc.sync.dma_start(out=QX[d:2 * d, nbs], in_=qlt)

        def k_proj(nb):
            nbs = ds(nb * 512, 512)
            kth_t = ktstream.tile([P, KC, 512], f16, tag="kth", name="kth")
            nc.sync.dma_start(out=kth_t,
                              in_=kth_d[:, nbs].rearrange("(c p) x -> p c x", p=P))
            ktl_t = ktstream.tile([P, KC, 512], f16, tag="ktl", name="ktl")
            nc.sync.dma_start(out=ktl_t,
                              in_=ktl_d[:, nbs].rearrange("(c p) x -> p c x", p=P))
            ps_k = pk_pool.tile([d, 512], f32)
            terms = [(wkh_sb, kth_t), (wkh_sb, ktl_t), (wkl_sb, kth_t)]
            for i, (w, x) in enumerate(terms):
                for kc in range(KC):
                    nc.tensor.matmul(ps_k, lhsT=w[:, kc, :], rhs=x[:, kc, :],
                                     start=(i == 0 and kc == 0),
                                     stop=(i == 2 and kc == KC - 1))
            nc.scalar.copy(KH[0:d, nbs], ps_k)
            nc.vector.tensor_sub(KX[0:d, nbs], ps_k, KH[0:d, nbs])
            nc.sync.dma_start(out=KX[d:2 * d, nbs], in_=KH[0:d, nbs])

        def v_tile(mt):
            vt_t = pstream.tile([P, KC, P], f16, tag="vt", name="vt")
            nc.scalar.dma_start(out=vt_t,
                              in_=vt_d[:, ts(mt, P)].rearrange("(c p) x -> p c x", p=P))
            ps_v = pv_pool.tile([P, d], f32)
            for kc in range(KC):
                nc.tensor.matmul(ps_v, lhsT=vt_t[:, kc, :], rhs=wv_sb[:, kc, :],
                                 start=(kc == 0), stop=(kc == KC - 1))
            nc.vector.tensor_copy(v_sb[:, mt, 0:d], ps_v)

        # DMA queue order = program order: kt0, kt1, qt0, kt2..kt7, vt0..7,
        # qt1..7, vt8..31.  Era-0's critical deps (all K, chunk-0 q, early v)
        # land first; stats(0) items fire as their key blocks complete.
        k_proj(0)
        k_proj(1)
        q_proj(0)
        for j in range(NTC):
            stats_item(0, j * NTC + 0, st0_pool, "st0")
        for nb in range(2, NB):
            k_proj(nb)
            if nb % 2 == 1:
                i = nb // 2
                for j in range(NTC):
                    stats_item(0, j * NTC + i, st0_pool, "st0")
        for mt in range(0, 8):
            v_tile(mt)
        for nb in range(1, NB):
            q_proj(nb)
        # odd-era copy of q (separate tile so era c+1's rowmax scatter never
        # WARs era c's score matmul reads)
        nc.sync.dma_start(out=QH_od[0:d, :], in_=QH_ev[0:d, :])
        for mt in range(8, NT):
            v_tile(mt)

    # ---- P2: stats (chunk c+1) interleaved with main (chunk c) ----
    # One shared PSUM ring (tag "big") feeds both the main-score exp (ACT)
    # and the stats reduce (DVE): 3 slots decouple producers from whichever
    # consumer is momentarily behind. 6 + 1 (av) + 1 (wo) = 8 banks.
    with tc.tile_pool(name="big_ps_pool", bufs=3, space="PSUM") as big_pool, \
         tc.tile_pool(name="av_ps_pool", bufs=1, space="PSUM") as av_pool, \
         tc.tile_pool(name="wo_ps_pool", bufs=1, space="PSUM") as wo_pool, \
         tc.tile_pool(name="att_pool", bufs=6) as att_pool:

        def wo_tile(t):
            """Output-projection for n-tile t, scaled by 1/sumexp on eviction."""
            wops = wo_pool.tile([P, dim], f32)
            nc.tensor.matmul(wops, lhsT=outT[:, ts(t, P)], rhs=wo_sb,
                             start=True, stop=True)
            o_sb = att_pool.tile([P, dim], f16, tag="o_sb", name="o_sb")
            nc.scalar.mul(o_sb, wops, rinv[:, t:t + 1])
            nc.sync.dma_start(out=out_d[ts(t, P), :], in_=o_sb)

        # item k -> group: 16 items over groups 0..11, four groups carry 2
        _item_group = [0, 0, 1, 2, 3, 3, 4, 5, 6, 6, 7, 8, 9, 9, 10, 11]
        stats_sched = {}
        for _k, _g in enumerate(_item_group):
            stats_sched.setdefault(_g, []).append(_k)

        def era(c):
            """Main pass for chunk c; stats for chunk c+1 interleaved."""
            At = QH_ev if c % 2 == 0 else QH_od
            cs = ds(c * CH, CH)
            r65 = At[:, cs]   # [65, 512], row 64 = rowmax
            rx = QX[:, cs]    # [128, 512]: q_hi / q_lo
            av_ps = av_pool.tile([d + 1, 512], f32, tag="av")
            att_fifo = []  # (att_tile, g) awaiting attn.V, deferred 2 groups

            def emit_av(att_t, g):
                nc.tensor.matmul(av_ps, lhsT=v_sb[:, 2 * g, :], rhs=att_t[:, 0:512],
                                 start=(g == 0), stop=False)
                nc.tensor.matmul(av_ps, lhsT=v_sb[:, 2 * g + 1, :], rhs=att_t[:, 512:1024],
                                 start=False, stop=(g == GRP - 1))

            for g in range(GRP):
                # 16 stats items spread over the first 12 groups (4 carry 2):
                # even DVE load across the era, and the last rowmax scatter
                # still lands ~4 groups before era c+1 reads it
                if c + 1 < NCH:
                    for k in stats_sched.get(g, ()):
                        stats_item(c + 1, k, big_pool, "big")
                sc_ps = big_pool.tile([P, 1024], f32, tag="big", name="sc_ps")
                att_t = att_pool.tile([P, 1024], bf16, tag="att")
                nc.tensor.matmul(sc_ps[:, 0:512], lhsT=KH[:, ts(2 * g, P)], rhs=r65,
                                 start=True, stop=False)
                nc.tensor.matmul(sc_ps[:, 512:1024], lhsT=KH[:, ts(2 * g + 1, P)], rhs=r65,
                                 start=True, stop=False)
                nc.tensor.matmul(sc_ps[:, 0:512], lhsT=KX[:, ts(2 * g, P)], rhs=rx,
                                 start=False, stop=True)
                nc.tensor.matmul(sc_ps[:, 512:1024], lhsT=KX[:, ts(2 * g + 1, P)], rhs=rx,
                                 start=False, stop=True)
                nc.scalar.activation(att_t, sc_ps, AF.Exp)
                # defer attn.V two groups so the exp it reads is long done
                att_fifo.append((att_t, g))
                if len(att_fifo) > 2:
                    emit_av(*att_fifo.pop(0))
                # previous chunk's output projection, one tile per 4 groups
                if c >= 1 and g % NTC == 2:
                    wo_tile((c - 1) * NTC + g // NTC)
            for item in att_fifo:
                emit_av(*item)
            # evict attn_u @ v and the sumexp row, then gather denominators
            nc.scalar.copy(outT[:, cs], av_ps[0:d, :])
            nc.scalar.copy(sumx[:, cs], av_ps[d:d + 1, :])
            for jj in range(NTC):
                nc.sync.dma_start(out=rsum[:, c * NTC + jj:c * NTC + jj + 1],
                                  in_=sumx[:, ds(c * CH + jj * P, P)])
            nc.vector.reciprocal(rinv[:, ds(c * NTC, NTC)], rsum[:, ds(c * NTC, NTC)])

        for c in range(NCH):
            era(c)

        # ---- P3 tail: last chunk's output projection ----
        for t in range((NCH - 1) * NTC, NT):
            wo_tile(t)


def make_in_maps(Q, K, V, Wq, Wk, Wv, Wo):
    """Host-side sharding: transpose activations, slice weights per head."""
    import ml_dtypes

    scale = 1.0 / np.sqrt(Wq.shape[-1])

    def _hilo(x):
        hi = x.astype(np.float16)
        lo = (x - hi.astype(np.float32)).astype(np.float16)
        return np.ascontiguousarray(hi), np.ascontiguousarray(lo)

    QTH, QTL = _hilo(np.asarray(Q).T.astype(np.float32))
    KTH, KTL = _hilo(np.asarray(K).T.astype(np.float32))
    VT = np.ascontiguousarray(np.asarray(V).T.astype(np.float16))
    d = Wq.shape[-1]
    in_maps = []
    for h in range(Wq.shape[0]):
        wqh, wql = _hilo(Wq[h].astype(np.float32) * scale)
        wkh, wkl = _hilo(Wk[h].astype(np.float32))
        in_maps.append({
            "QTH": QTH, "QTL": QTL, "KTH": KTH, "KTL": KTL, "VT": VT,
            "wqh": wqh, "wql": wql, "wkh": wkh, "wkl": wkl,
            "wv": np.ascontiguousarray(Wv[h].astype(np.float16)),
            "wo": np.ascontiguousarray(Wo[h * d:(h + 1) * d, :].astype(np.float16)),
        })
    return in_maps


_CACHE = {}


def _build_and_compile(n=N, dim=DIM, d=D, num_cores=H, repeats=1):
    import concourse.bass as bass
    import concourse.mybir as mybir
    import concourse.tile as tile
    from concourse import bacc

    key = (n, dim, d, num_cores, repeats)
    if key in _CACHE:
        return _CACHE[key]
    nc = bacc.Bacc("TRN2", target_bir_lowering=False, debug=False,
                   num_devices=num_cores)
    f32 = mybir.dt.float32
    f32r = mybir.dt.float32r
    f16 = mybir.dt.float16
    bf16 = mybir.dt.bfloat16
    ins = {}
    for name in ("QTH", "QTL", "KTH", "KTL"):
        ins[name] = nc.dram_tensor(name, [dim, n], f16, kind="ExternalInput").ap()
    ins["VT"] = nc.dram_tensor("VT", [dim, n], f16, kind="ExternalInput").ap()
    for name in ("wqh", "wql", "wkh", "wkl"):
        ins[name] = nc.dram_tensor(name, [dim, d], f16, kind="ExternalInput").ap()
    ins["wv"] = nc.dram_tensor("wv", [dim, d], f16, kind="ExternalInput").ap()
    ins["wo"] = nc.dram_tensor("wo", [d, dim], f16, kind="ExternalInput").ap()
    outs = {"out": nc.dram_tensor("out", [n, dim], f16, kind="ExternalOutput").ap()}
    with tile.TileContext(nc) as tc:
        for _rep in range(repeats):
            with ExitStack() as ctx:
                build_head_kernel(ctx, tc, outs, ins, n=n, dim=dim, d=d)
    nc.compile()
    _CACHE[key] = nc
    return nc


def run_on_hw(in_maps, trace=False, **kwargs):
    from concourse.bass_utils import run_bass_kernel_spmd

    nc = _build_and_compile(num_cores=len(in_maps))
    return run_bass_kernel_spmd(nc, in_maps, core_ids=list(range(len(in_maps))),
                                trace=trace, **kwargs)


def kernel(Q, K, V, Wq, Wk, Wv, Wo):
    in_maps = make_in_maps(np.asarray(Q), np.asarray(K), np.asarray(V),
                           np.asarray(Wq), np.asarray(Wk), np.asarray(Wv),
                           np.asarray(Wo))
    res = run_on_hw(in_maps)
    out = np.zeros((N, DIM), dtype=np.float64)
    for r in res.results:
        out += r["out"].astype(np.float64)
    return out.astype(np.float32)


if __name__ == "__main__":
    rng = np.random.default_rng(0)
    inputs = {
        "Q": rng.standard_normal((N, DIM), dtype=np.float32),
        "K": rng.standard_normal((N, DIM), dtype=np.float32),
        "V": rng.standard_normal((N, DIM), dtype=np.float32),
        "Wq": rng.random((H, DIM, D), dtype=np.float32),
        "Wk": rng.random((H, DIM, D), dtype=np.float32),
        "Wv": rng.random((H, DIM, D), dtype=np.float32),
        "Wo": rng.random((DIM, DIM), dtype=np.float32),
    }
    out = kernel(**inputs)
    print(out.shape, out.dtype, np.abs(out).max())


# revision 25
# speedup vs baseline: 1.2299x; 1.0168x over previous
"""Multi-head attention on 8 Trainium2 NeuronCores (head-parallel).

Problem: Q,K,V [4096,512] fp32; Wq/Wk/Wv [8,512,64]; Wo [512,512].
  out = concat_h(softmax(QWq_h (KWk_h)^T / sqrt(64)) VWv_h) @ Wo

Sharding: one head per core. Each core computes its head end-to-end plus
its slice of the output projection (out_h @ Wo[64h:64h+64, :]); the host
sums the 8 partial [4096,512] outputs.

Per-core pipeline (n = 4096 queries, m = 4096 keys, d = 64):
  P1 (head): K projection + chunk-0 q projection + chunk-0 stats + the
      first half of the v tiles, all chasing the input DMA stream. Only
      the work era 0 actually needs lives here; every other projection is
      injected into the era pipeline so the PE never idles on a serial
      projection tail.
  Eras (one per 512-query chunk c):
      stats pass (fp16 hi*hi, for chunk c+1): natural-layout scores ->
        per-row max (DVE reduce over PSUM); row maxes are DMA-scattered
        into row 64 of the fp16 q operand. The max error (~|s|*2^-11)
        cancels exactly: softmax is shift-invariant and the denominator
        is computed from the same shifted weights.
      main pass (fp16 hi/lo, 2 matmuls per m-tile): transposed scores.
        Pass 1 is hi*hi with a 65th contraction row (k side = -1, q side
        = rowmax) so PSUM holds qk^T - rowmax directly; pass 2 fuses both
        cross terms as [k_lo;k_hi] x [q_hi;q_lo] at K=128. Only the
        lo*lo term (~|s|*2^-22) is dropped: scores here reach |s| ~ 1e4
        (the all-positive projection weights give q and k a large shared
        sign pattern), so ~2^-17 relative accuracy is required -- which
        also rules out single-pass fp32r (~2^-13) for scores AND for the
        projections (hence fp16 hi/lo 3-term projections).
      attn.V (bf16): accumulate outT [65, 512] in PSUM over all 32
        m-tiles; row 64 (ones column of v) is the softmax denominator.
        bf16 (not fp16) because exp(s - rowmax) can reach e^+12 from the
        stats/rowmax fp16 rounding.
      injected work: q projection for chunk c+2 (fp16 hi/lo 3-term), its
        hi/lo evictions + relocations, the odd-tile copy, the second half
        of the v tiles (era 0), and the PREVIOUS chunk's output
        projection (Wo, fp16), one n-tile per 4 groups, scaled by 1/sum
        on eviction. These share one spare PSUM bank (tag "aux").
  Tail: last chunk's Wo tiles.
"""

from contextlib import ExitStack

import numpy as np

N = 4096
DIM = 512
H = 8
D = 64
P = 128
CH = 512  # query columns per era (chunk)


def build_head_kernel(ctx, tc, outs, ins, n=N, dim=DIM, d=D):
    import concourse.bass as bass
    import concourse.mybir as mybir
    from concourse.bass import ts, ds

    nc = tc.nc
    f32 = mybir.dt.float32
    f16 = mybir.dt.float16
    bf16 = mybir.dt.bfloat16
    AF = mybir.ActivationFunctionType
    X = mybir.AxisListType.X

    KC = dim // P      # projection contraction chunks (4)
    NT = n // P        # 128-row tiles of n (= m tiles) (32)
    NCH = n // CH      # eras (8)
    NTC = CH // P      # n-tiles per era (4)
    NB = n // 512      # projection column blocks (8)
    GRP = NT // 2      # main groups per era, 2 m-tiles each (16)
    assert CH == 512 and n % 1024 == 0

    qth_d, qtl_d = ins["QTH"], ins["QTL"]
    kth_d, ktl_d = ins["KTH"], ins["KTL"]
    vt_d = ins["VT"]
    wqh_d, wql_d = ins["wqh"], ins["wql"]
    wkh_d, wkl_d = ins["wkh"], ins["wkl"]
    wv_d, wo_d = ins["wv"], ins["wo"]
    out_d = outs["out"]

    singles = ctx.enter_context(tc.tile_pool(name="singles", bufs=1))

    QH_ev = singles.tile([d + 1, n], f16)   # rows 0-63 q_hi; row 64 rowmax
    QH_od = singles.tile([d + 1, n], f16)
    KH = singles.tile([d + 1, n], f16)      # rows 0-63 k_hi; row 64 = -1
    QX = singles.tile([P, n], f16)          # rows 0-63 q_hi, 64-127 q_lo
    KX = singles.tile([P, n], f16)          # rows 0-63 k_lo, 64-127 k_hi
    v_sb = singles.tile([P, NT, d + 1], bf16)  # v tiles + ones column
    outT = singles.tile([d, n], bf16)       # attn_u @ v
    sumx = singles.tile([1, n], f32)        # softmax denominators
    rsum = singles.tile([P, NT], f32)       # sumexp gathered per n-tile
    rinv = singles.tile([P, NT], f32)
    wqh_sb = singles.tile([P, KC, d], f16)
    wql_sb = singles.tile([P, KC, d], f16)
    wkh_sb = singles.tile([P, KC, d], f16)
    wkl_sb = singles.tile([P, KC, d], f16)
    wv_sb = singles.tile([P, KC, d], f16)
    wo_sb = singles.tile([d, dim], f16)

    # q/v input streams live across eras: DMAs are issued ahead (P1 or the
    # preceding era) and the projection compute is injected later.
    qstream = ctx.enter_context(tc.tile_pool(name="qstream", bufs=3))
    vstream = ctx.enter_context(tc.tile_pool(name="vstream", bufs=34))
    nmax_pool = ctx.enter_context(tc.tile_pool(name="nmax_pool", bufs=6))

    qt_tiles = {}

    def q_dma(nb):
        nbs = ds(nb * 512, 512)
        qth_t = qstream.tile([P, KC, 512], f16, tag="qth", name="qth")
        nc.sync.dma_start(out=qth_t,
                          in_=qth_d[:, nbs].rearrange("(c p) x -> p c x", p=P))
        qtl_t = qstream.tile([P, KC, 512], f16, tag="qtl", name="qtl")
        nc.sync.dma_start(out=qtl_t,
                          in_=qtl_d[:, nbs].rearrange("(c p) x -> p c x", p=P))
        qt_tiles[nb] = (qth_t, qtl_t)

    def v_dma(mt):
        vt_t = vstream.tile([P, KC, P], f16, tag="vt", name="vt")
        nc.sync.dma_start(out=vt_t,
                            in_=vt_d[:, ts(mt, P)].rearrange("(c p) x -> p c x", p=P))
        return vt_t

    def q_proj_mms(nb, ps_q, lo, hi):
        """Projection matmul slots [lo, hi) of the 12 (term, kc) pairs."""
        qth_t, qtl_t = qt_tiles[nb]
        terms = [(wqh_sb, qth_t), (wqh_sb, qtl_t), (wql_sb, qth_t)]
        for s in range(lo, hi):
            i, kc = divmod(s, KC)
            w, x = terms[i]
            nc.tensor.matmul(ps_q, lhsT=w[:, kc, :], rhs=x[:, kc, :],
                             start=(s == 0), stop=(s == 3 * KC - 1))

    def q_evict(nb, ps_q):
        nbs = ds(nb * 512, 512)
        nc.scalar.copy(QH_ev[0:d, nbs], ps_q)
        nc.scalar.copy(QX[0:d, nbs], ps_q)
        qlt = qstream.tile([d, 512], f16, tag="qlt", name="qlt")
        nc.vector.tensor_sub(qlt, ps_q, QX[0:d, nbs])
        nc.sync.dma_start(out=QX[d:2 * d, nbs], in_=qlt)
        # odd-tile copy of this q chunk (row 64 is scattered separately)
        nc.sync.dma_start(out=QH_od[0:d, nbs], in_=QH_ev[0:d, nbs])
        del qt_tiles[nb]

    def v_proj(mt, vt_t, pool, tag):
        ps_v = pool.tile([P, dim], f32, tag=tag, name="ps_v")
        for kc in range(KC):
            nc.tensor.matmul(ps_v[:, 0:d], lhsT=vt_t[:, kc, :], rhs=wv_sb[:, kc, :],
                             start=(kc == 0), stop=(kc == KC - 1))
        nc.vector.tensor_copy(v_sb[:, mt, 0:d], ps_v[:, 0:d])

    nmax_tiles = {}

    def stats_item(c, k, pool, tag):
        """Stats for chunk c, item k: n-tile j = k//4 vs key block i = k%4."""
        j, i = divmod(k, NTC)
        gt = c * NTC + j  # global n-tile
        if i == 0:
            nmax_tiles[j] = nmax_pool.tile([P, NTC], f32, tag="nmax",
                                           name="nmax")
        st_ps = pool.tile([P, 1024], f32, tag=tag, name="st_ps")
        nc.tensor.matmul(st_ps[:, 0:512], lhsT=QH_ev[0:d, ts(gt, P)],
                         rhs=KH[0:d, ds(i * 1024, 512)], start=True, stop=True)
        nc.tensor.matmul(st_ps[:, 512:1024], lhsT=QH_ev[0:d, ts(gt, P)],
                         rhs=KH[0:d, ds(i * 1024 + 512, 512)], start=True, stop=True)
        nc.vector.reduce_max(nmax_tiles[j][:, i:i + 1], st_ps, axis=X)
        if i == NTC - 1:
            cm = nmax_pool.tile([P, 1], f16, tag="cm", name="cm")
            nc.vector.reduce_max(cm, nmax_tiles[j], axis=X)
            At = QH_ev if c % 2 == 0 else QH_od
            # scatter per-row maxes into row 64: column n = c*CH + j*P + row
            nc.sync.dma_start(out=At[d:d + 1, ds(c * CH + j * P, P)], in_=cm)

    # ---- P1 head: K + chunk-0 q + chunk-0 stats + first-half v tiles ----
    with tc.tile_pool(name="kstream", bufs=4) as kstream, \
         tc.tile_pool(name="pq_ps", bufs=1, space="PSUM") as pq_pool, \
         tc.tile_pool(name="pk_ps", bufs=1, space="PSUM") as pk_pool, \
         tc.tile_pool(name="pv_ps", bufs=1, space="PSUM") as pv_pool, \
         tc.tile_pool(name="st0_ps", bufs=2, space="PSUM") as st0_pool:

        # weights on the ACT hwdge queue so kth0 is the first SP transfer
        def _load_w(w_sb, w_d):
            nc.scalar.dma_start(out=w_sb, in_=w_d.rearrange("(c p) e -> p c e", p=P))
        _load_w(wkh_sb, wkh_d)
        _load_w(wkl_sb, wkl_d)
        _load_w(wqh_sb, wqh_d)
        _load_w(wql_sb, wql_d)
        _load_w(wv_sb, wv_d)
        nc.scalar.dma_start(out=wo_sb, in_=wo_d)
        nc.vector.memset(KH[d:d + 1, :], -1.0)
        nc.vector.memset(v_sb[:, :, d:d + 1], 1.0)

        def k_proj(nb):
            nbs = ds(nb * 512, 512)
            kth_t = kstream.tile([P, KC, 512], f16, tag="kth", name="kth")
            nc.sync.dma_start(out=kth_t,
                              in_=kth_d[:, nbs].rearrange("(c p) x -> p c x", p=P))
            ktl_t = kstream.tile([P, KC, 512], f16, tag="ktl", name="ktl")
            nc.sync.dma_start(out=ktl_t,
                              in_=ktl_d[:, nbs].rearrange("(c p) x -> p c x", p=P))
            ps_k = pk_pool.tile([d, 512], f32)
            terms = [(wkh_sb, kth_t), (wkh_sb, ktl_t), (wkl_sb, kth_t)]
            for i, (w, x) in enumerate(terms):
                for kc in range(KC):
                    nc.tensor.matmul(ps_k, lhsT=w[:, kc, :], rhs=x[:, kc, :],
                                     start=(i == 0 and kc == 0),
                                     stop=(i == 2 and kc == KC - 1))
            nc.scalar.copy(KH[0:d, nbs], ps_k)
            nc.vector.tensor_sub(KX[0:d, nbs], ps_k, KH[0:d, nbs])
            nc.sync.dma_start(out=KX[d:2 * d, nbs], in_=KH[0:d, nbs])

        # SP-queue DMA order is the P1 critical path: qt0 (stats needs
        # chunk-0 q), then all of K, then qt1/qt2; VT rides the ACT queue.
        q_dma(0)
        ps_q0 = pq_pool.tile([d, 512], f32)
        q_proj_mms(0, ps_q0, 0, 12)
        q_evict(0, ps_q0)
        k_proj(0)
        k_proj(1)
        for j in range(NTC):
            stats_item(0, j * NTC + 0, st0_pool, "st0")
        for nb in range(2, NB):
            k_proj(nb)
            if nb % 2 == 1:
                i = nb // 2
                for j in range(NTC):
                    stats_item(0, j * NTC + i, st0_pool, "st0")
        q_dma(1)
        q_dma(2)
        for nb in (1, 2):
            ps_qx = pq_pool.tile([d, 512], f32, name="ps_qx")
            q_proj_mms(nb, ps_qx, 0, 12)
            q_evict(nb, ps_qx)
        # v DMAs all issued here (ACT queue, behind the small weight loads);
        # tiles 0-15 projected now, 16-31 injected into era 0
        vt_tiles = [v_dma(mt) for mt in range(NT)]
        for mt in range(16):
            v_proj(mt, vt_tiles[mt], pv_pool, "pv")

    # ---- eras: main + stats(c+1) + injected projections + Wo(c-1) ----
    with tc.tile_pool(name="big_ps_pool", bufs=3, space="PSUM") as big_pool, \
         tc.tile_pool(name="av_ps_pool", bufs=1, space="PSUM") as av_pool, \
         tc.tile_pool(name="aux_ps_pool", bufs=1, space="PSUM") as aux_pool, \
         tc.tile_pool(name="att_pool", bufs=6) as att_pool:

        def wo_tile(t, pool=None, tag="aux"):
            """Output-projection for n-tile t, scaled by 1/sumexp on eviction."""
            wops = (pool or aux_pool).tile([P, dim], f32, tag=tag, name="wops")
            nc.tensor.matmul(wops, lhsT=outT[:, ts(t, P)], rhs=wo_sb,
                             start=True, stop=True)
            o_sb = att_pool.tile([P, dim], f16, tag="o_sb", name="o_sb")
            nc.scalar.mul(o_sb, wops, rinv[:, t:t + 1])
            nc.sync.dma_start(out=out_d[ts(t, P), :], in_=o_sb)

        # stats item k of chunk c+1 runs at group _item_group[k]+2 of era c:
        # spread for even DVE load, finishing with >= 1.5 groups of slack
        _item_group = [0, 0, 1, 2, 3, 3, 4, 5, 6, 6, 7, 8, 9, 9, 10, 11]
        stats_sched = {}
        for _k, _g in enumerate(_item_group):
            stats_sched.setdefault(_g + 2, []).append(_k)

        def era(c):
            """Main pass for chunk c; stats(c+1), q_proj(c+2), Wo(c-1) woven in."""
            At = QH_ev if c % 2 == 0 else QH_od
            cs = ds(c * CH, CH)
            r65 = At[:, cs]   # [65, 512], row 64 = rowmax
            rx = QX[:, cs]    # [128, 512]: q_hi / q_lo
            av_ps = av_pool.tile([d + 1, 512], f32, tag="av")
            att_fifo = []  # (att_tile, g) awaiting attn.V, deferred 2 groups
            ps_q = None

            def emit_av(att_t, g):
                nc.tensor.matmul(av_ps, lhsT=v_sb[:, 2 * g, :], rhs=att_t[:, 0:512],
                                 start=(g == 0), stop=False)
                nc.tensor.matmul(av_ps, lhsT=v_sb[:, 2 * g + 1, :], rhs=att_t[:, 512:1024],
                                 start=False, stop=(g == GRP - 1))

            for g in range(GRP):
                if c + 1 < NCH:
                    for k in stats_sched.get(g, ()):
                        stats_item(c + 1, k, big_pool, "big")
                sc_ps = big_pool.tile([P, 1024], f32, tag="big", name="sc_ps")
                att_t = att_pool.tile([P, 1024], bf16, tag="att")
                nc.tensor.matmul(sc_ps[:, 0:512], lhsT=KH[:, ts(2 * g, P)], rhs=r65,
                                 start=True, stop=False)
                nc.tensor.matmul(sc_ps[:, 512:1024], lhsT=KH[:, ts(2 * g + 1, P)], rhs=r65,
                                 start=True, stop=False)
                nc.tensor.matmul(sc_ps[:, 0:512], lhsT=KX[:, ts(2 * g, P)], rhs=rx,
                                 start=False, stop=True)
                nc.tensor.matmul(sc_ps[:, 512:1024], lhsT=KX[:, ts(2 * g + 1, P)], rhs=rx,
                                 start=False, stop=True)
                nc.scalar.activation(att_t, sc_ps, AF.Exp)
                # injected work sits AFTER this group's score matmuls so an
                # aux-bank wait never head-of-line blocks the PE stream
                if 1 <= c and c + 2 < NCH:
                    if g == 0:
                        q_dma(c + 2)
                        ps_q = aux_pool.tile([P, dim], f32, tag="aux", name="ps_q")
                    if g < 3:
                        q_proj_mms(c + 2, ps_q[0:d, :], 4 * g, 4 * (g + 1))
                    elif g == 3:
                        q_evict(c + 2, ps_q[0:d, :])
                if c == 0:
                    # second-half v tiles (tile g+16 is needed by the attn.V
                    # of group (g+16)/2 + 2, always comfortably later)
                    v_proj(g + 16, vt_tiles[g + 16], aux_pool, "aux")
                # defer attn.V two groups so the exp it reads is long done
                att_fifo.append((att_t, g))
                if len(att_fifo) > 2:
                    emit_av(*att_fifo.pop(0))
                # previous chunk's output projection, one tile per 4 groups
                if c >= 1 and g % NTC == 2:
                    wo_tile((c - 1) * NTC + g // NTC)
            for item in att_fifo:
                emit_av(*item)
            # evict attn_u @ v and the sumexp row, then gather denominators
            nc.scalar.copy(outT[:, cs], av_ps[0:d, :])
            nc.scalar.copy(sumx[:, cs], av_ps[d:d + 1, :])
            for jj in range(NTC):
                nc.sync.dma_start(out=rsum[:, c * NTC + jj:c * NTC + jj + 1],
                                  in_=sumx[:, ds(c * CH + jj * P, P)])
            nc.vector.reciprocal(rinv[:, ds(c * NTC, NTC)], rsum[:, ds(c * NTC, NTC)])

        for c in range(NCH):
            era(c)

        # ---- tail: last chunk's output projection (alternating PSUM
        # rings so the four tiles pipeline instead of serializing) ----
        for t in range((NCH - 1) * NTC, NT):
            if t % 2 == 0:
                wo_tile(t)
            else:
                wo_tile(t, big_pool, "big")


def make_in_maps(Q, K, V, Wq, Wk, Wv, Wo):
    """Host-side sharding: transpose activations, slice weights per head."""
    scale = 1.0 / np.sqrt(Wq.shape[-1])

    def _hilo(x):
        hi = x.astype(np.float16)
        lo = (x - hi.astype(np.float32)).astype(np.float16)
        return np.ascontiguousarray(hi), np.ascontiguousarray(lo)

    QTH, QTL = _hilo(np.asarray(Q).T.astype(np.float32))
    KTH, KTL = _hilo(np.asarray(K).T.astype(np.float32))
    VT = np.ascontiguousarray(np.asarray(V).T.astype(np.float16))
    d = Wq.shape[-1]
    in_maps = []
    for h in range(Wq.shape[0]):
        wqh, wql = _hilo(Wq[h].astype(np.float32) * scale)
        wkh, wkl = _hilo(Wk[h].astype(np.float32))
        in_maps.append({
            "QTH": QTH, "QTL": QTL, "KTH": KTH, "KTL": KTL, "VT": VT,
            "wqh": wqh, "wql": wql, "wkh": wkh, "wkl": wkl,
            "wv": np.ascontiguousarray(Wv[h].astype(np.float16)),
            "wo": np.ascontiguousarray(Wo[h * d:(h + 1) * d, :].astype(np.float16)),
        })
    return in_maps


_CACHE = {}


def _build_and_compile(n=N, dim=DIM, d=D, num_cores=H, repeats=1):
    import concourse.bass as bass
    import concourse.mybir as mybir
    import concourse.tile as tile
    from concourse import bacc

    key = (n, dim, d, num_cores, repeats)
    if key in _CACHE:
        return _CACHE[key]
    nc = bacc.Bacc("TRN2", target_bir_lowering=False, debug=False,
                   num_devices=num_cores)
    f32 = mybir.dt.float32
    f16 = mybir.dt.float16
    ins = {}
    for name in ("QTH", "QTL", "KTH", "KTL"):
        ins[name] = nc.dram_tensor(name, [dim, n], f16, kind="ExternalInput").ap()
    ins["VT"] = nc.dram_tensor("VT", [dim, n], f16, kind="ExternalInput").ap()
    for name in ("wqh", "wql", "wkh", "wkl"):
        ins[name] = nc.dram_tensor(name, [dim, d], f16, kind="ExternalInput").ap()
    ins["wv"] = nc.dram_tensor("wv", [dim, d], f16, kind="ExternalInput").ap()
    ins["wo"] = nc.dram_tensor("wo", [d, dim], f16, kind="ExternalInput").ap()
    outs = {"out": nc.dram_tensor("out", [n, dim], f16, kind="ExternalOutput").ap()}
    with tile.TileContext(nc) as tc:
        for _rep in range(repeats):
            with ExitStack() as ctx:
                build_head_kernel(ctx, tc, outs, ins, n=n, dim=dim, d=d)
    nc.compile()
    _CACHE[key] = nc
    return nc


def run_on_hw(in_maps, trace=False, **kwargs):
    from concourse.bass_utils import run_bass_kernel_spmd

    nc = _build_and_compile(num_cores=len(in_maps))
    return run_bass_kernel_spmd(nc, in_maps, core_ids=list(range(len(in_maps))),
                                trace=trace, **kwargs)


def kernel(Q, K, V, Wq, Wk, Wv, Wo):
    in_maps = make_in_maps(np.asarray(Q), np.asarray(K), np.asarray(V),
                           np.asarray(Wq), np.asarray(Wk), np.asarray(Wv),
                           np.asarray(Wo))
    res = run_on_hw(in_maps)
    out = np.zeros((N, DIM), dtype=np.float64)
    for r in res.results:
        out += r["out"].astype(np.float64)
    return out.astype(np.float32)


if __name__ == "__main__":
    rng = np.random.default_rng(0)
    inputs = {
        "Q": rng.standard_normal((N, DIM), dtype=np.float32),
        "K": rng.standard_normal((N, DIM), dtype=np.float32),
        "V": rng.standard_normal((N, DIM), dtype=np.float32),
        "Wq": rng.random((H, DIM, D), dtype=np.float32),
        "Wk": rng.random((H, DIM, D), dtype=np.float32),
        "Wv": rng.random((H, DIM, D), dtype=np.float32),
        "Wo": rng.random((DIM, DIM), dtype=np.float32),
    }
    out = kernel(**inputs)
    print(out.shape, out.dtype, np.abs(out).max())


# revision 28
# speedup vs baseline: 1.2459x; 1.0130x over previous
"""Multi-head attention on 8 Trainium2 NeuronCores (head-parallel).

Problem: Q,K,V [4096,512] fp32; Wq/Wk/Wv [8,512,64]; Wo [512,512].
  out = concat_h(softmax(QWq_h (KWk_h)^T / sqrt(64)) VWv_h) @ Wo

Sharding: one head per core. Each core computes its head end-to-end plus
its slice of the output projection (out_h @ Wo[64h:64h+64, :]); the host
sums the 8 partial [4096,512] outputs.

Per-core pipeline (n = 4096 queries, m = 4096 keys, d = 64):
  P1 (head): K projection + chunk-0 q projection + chunk-0 stats + the
      first half of the v tiles, all chasing the input DMA stream. Only
      the work era 0 actually needs lives here; every other projection is
      injected into the era pipeline so the PE never idles on a serial
      projection tail.
  Eras (one per 512-query chunk c):
      stats pass (fp16 hi*hi, for chunk c+1): natural-layout scores ->
        per-row max (DVE reduce over PSUM); row maxes are DMA-scattered
        into row 64 of the fp16 q operand. The max error (~|s|*2^-11)
        cancels exactly: softmax is shift-invariant and the denominator
        is computed from the same shifted weights.
      main pass (fp16 hi/lo, 2 matmuls per m-tile): transposed scores.
        Pass 1 is hi*hi with a 65th contraction row (k side = -1, q side
        = rowmax) so PSUM holds qk^T - rowmax directly; pass 2 fuses both
        cross terms as [k_lo;k_hi] x [q_hi;q_lo] at K=128. Only the
        lo*lo term (~|s|*2^-22) is dropped: scores here reach |s| ~ 1e4
        (the all-positive projection weights give q and k a large shared
        sign pattern), so ~2^-17 relative accuracy is required -- which
        also rules out single-pass fp32r (~2^-13) for scores AND for the
        projections (hence fp16 hi/lo 3-term projections).
      attn.V (bf16): accumulate outT [65, 512] in PSUM over all 32
        m-tiles; row 64 (ones column of v) is the softmax denominator.
        bf16 (not fp16) because exp(s - rowmax) can reach e^+12 from the
        stats/rowmax fp16 rounding.
      injected work: q projection for chunk c+2 (fp16 hi/lo 3-term), its
        hi/lo evictions + relocations, the odd-tile copy, the second half
        of the v tiles (era 0), and the PREVIOUS chunk's output
        projection (Wo, fp16), one n-tile per 4 groups, scaled by 1/sum
        on eviction. These share one spare PSUM bank (tag "aux").
  Tail: last chunk's Wo tiles.
"""

from contextlib import ExitStack

import numpy as np

N = 4096
DIM = 512
H = 8
D = 64
P = 128
CH = 512  # query columns per era (chunk)


def build_head_kernel(ctx, tc, outs, ins, n=N, dim=DIM, d=D):
    import concourse.bass as bass
    import concourse.mybir as mybir
    from concourse.bass import ts, ds

    nc = tc.nc
    f32 = mybir.dt.float32
    f16 = mybir.dt.float16
    bf16 = mybir.dt.bfloat16
    AF = mybir.ActivationFunctionType
    X = mybir.AxisListType.X

    KC = dim // P      # projection contraction chunks (4)
    NT = n // P        # 128-row tiles of n (= m tiles) (32)
    NCH = n // CH      # eras (8)
    NTC = CH // P      # n-tiles per era (4)
    NB = n // 512      # projection column blocks (8)
    GRP = NT // 2      # main groups per era, 2 m-tiles each (16)
    assert CH == 512 and n % 1024 == 0

    qth_d, qtl_d = ins["QTH"], ins["QTL"]
    kth_d, ktl_d = ins["KTH"], ins["KTL"]
    vt_d = ins["VT"]
    wqh_d, wql_d = ins["wqh"], ins["wql"]
    wkh_d, wkl_d = ins["wkh"], ins["wkl"]
    wv_d, wo_d = ins["wv"], ins["wo"]
    out_d = outs["out"]

    singles = ctx.enter_context(tc.tile_pool(name="singles", bufs=1))

    QH_ev = singles.tile([d + 1, n], f16)   # rows 0-63 q_hi; row 64 rowmax
    QH_od = singles.tile([d + 1, n], f16)
    KH = singles.tile([d + 1, n], f16)      # rows 0-63 k_hi; row 64 = -1
    QX = singles.tile([P, n], f16)          # rows 0-63 q_hi, 64-127 q_lo
    KX = singles.tile([P, n], f16)          # rows 0-63 k_lo, 64-127 k_hi
    v_sb = singles.tile([P, NT, d + 1], bf16)  # v tiles + ones column
    outT = singles.tile([d, n], bf16)       # attn_u @ v
    sumx = singles.tile([1, n], f32)        # softmax denominators
    rsum = singles.tile([P, NT], f32)       # sumexp gathered per n-tile
    rinv = singles.tile([P, NT], f32)
    wqh_sb = singles.tile([P, KC, d], f16)
    wql_sb = singles.tile([P, KC, d], f16)
    wkh_sb = singles.tile([P, KC, d], f16)
    wkl_sb = singles.tile([P, KC, d], f16)
    wv_sb = singles.tile([P, KC, d], f16)
    wo_sb = singles.tile([d, dim], f16)

    # q/v input streams live across eras: DMAs are issued ahead (P1 or the
    # preceding era) and the projection compute is injected later.
    qstream = ctx.enter_context(tc.tile_pool(name="qstream", bufs=3))
    vstream = ctx.enter_context(tc.tile_pool(name="vstream", bufs=34))
    nmax_pool = ctx.enter_context(tc.tile_pool(name="nmax_pool", bufs=6))

    qt_tiles = {}

    def q_dma(nb):
        nbs = ds(nb * 512, 512)
        qth_t = qstream.tile([P, KC, 512], f16, tag="qth", name="qth")
        nc.sync.dma_start(out=qth_t,
                          in_=qth_d[:, nbs].rearrange("(c p) x -> p c x", p=P))
        qtl_t = qstream.tile([P, KC, 512], f16, tag="qtl", name="qtl")
        nc.sync.dma_start(out=qtl_t,
                          in_=qtl_d[:, nbs].rearrange("(c p) x -> p c x", p=P))
        qt_tiles[nb] = (qth_t, qtl_t)

    def v_dma(mt):
        vt_t = vstream.tile([P, KC, P], f16, tag="vt", name="vt")
        nc.sync.dma_start(out=vt_t,
                            in_=vt_d[:, ts(mt, P)].rearrange("(c p) x -> p c x", p=P))
        return vt_t

    def q_proj_mms(nb, ps_q, lo, hi):
        """Projection matmul slots [lo, hi) of the 12 (term, kc) pairs."""
        qth_t, qtl_t = qt_tiles[nb]
        terms = [(wqh_sb, qth_t), (wqh_sb, qtl_t), (wql_sb, qth_t)]
        for s in range(lo, hi):
            i, kc = divmod(s, KC)
            w, x = terms[i]
            nc.tensor.matmul(ps_q, lhsT=w[:, kc, :], rhs=x[:, kc, :],
                             start=(s == 0), stop=(s == 3 * KC - 1))

    def q_evict(nb, ps_q):
        nbs = ds(nb * 512, 512)
        nc.scalar.copy(QH_ev[0:d, nbs], ps_q)
        nc.scalar.copy(QX[0:d, nbs], ps_q)
        qlt = qstream.tile([d, 512], f16, tag="qlt", name="qlt")
        nc.vector.tensor_sub(qlt, ps_q, QX[0:d, nbs])
        nc.sync.dma_start(out=QX[d:2 * d, nbs], in_=qlt)
        # odd-tile copy of this q chunk (row 64 is scattered separately)
        nc.sync.dma_start(out=QH_od[0:d, nbs], in_=QH_ev[0:d, nbs])
        del qt_tiles[nb]

    def v_proj(mt, vt_t, pool, tag):
        ps_v = pool.tile([P, dim], f32, tag=tag, name="ps_v")
        for kc in range(KC):
            nc.tensor.matmul(ps_v[:, 0:d], lhsT=vt_t[:, kc, :], rhs=wv_sb[:, kc, :],
                             start=(kc == 0), stop=(kc == KC - 1))
        nc.vector.tensor_copy(v_sb[:, mt, 0:d], ps_v[:, 0:d])

    nmax_tiles = {}

    def stats_item(c, k, pool, tag):
        """Stats for chunk c, item k: n-tile j = k//4 vs key block i = k%4."""
        j, i = divmod(k, NTC)
        gt = c * NTC + j  # global n-tile
        if i == 0:
            nmax_tiles[j] = nmax_pool.tile([P, NTC], f32, tag="nmax",
                                           name="nmax")
        st_ps = pool.tile([P, 1024], f32, tag=tag, name="st_ps")
        nc.tensor.matmul(st_ps[:, 0:512], lhsT=QH_ev[0:d, ts(gt, P)],
                         rhs=KH[0:d, ds(i * 1024, 512)], start=True, stop=True)
        nc.tensor.matmul(st_ps[:, 512:1024], lhsT=QH_ev[0:d, ts(gt, P)],
                         rhs=KH[0:d, ds(i * 1024 + 512, 512)], start=True, stop=True)
        nc.vector.reduce_max(nmax_tiles[j][:, i:i + 1], st_ps, axis=X)
        if i == NTC - 1:
            cm = nmax_pool.tile([P, 1], f16, tag="cm", name="cm")
            nc.vector.reduce_max(cm, nmax_tiles[j], axis=X)
            At = QH_ev if c % 2 == 0 else QH_od
            # scatter per-row maxes into row 64: column n = c*CH + j*P + row
            nc.sync.dma_start(out=At[d:d + 1, ds(c * CH + j * P, P)], in_=cm)

    # ---- P1 head: K + chunk-0 q + chunk-0 stats + first-half v tiles ----
    with tc.tile_pool(name="kstream", bufs=4) as kstream, \
         tc.tile_pool(name="pq_ps", bufs=1, space="PSUM") as pq_pool, \
         tc.tile_pool(name="pk_ps", bufs=1, space="PSUM") as pk_pool, \
         tc.tile_pool(name="pv_ps", bufs=1, space="PSUM") as pv_pool, \
         tc.tile_pool(name="st0_ps", bufs=2, space="PSUM") as st0_pool:

        # weights on the ACT hwdge queue so kth0 is the first SP transfer
        def _load_w(w_sb, w_d):
            nc.scalar.dma_start(out=w_sb, in_=w_d.rearrange("(c p) e -> p c e", p=P))
        _load_w(wkh_sb, wkh_d)
        _load_w(wkl_sb, wkl_d)
        _load_w(wqh_sb, wqh_d)
        _load_w(wql_sb, wql_d)
        _load_w(wv_sb, wv_d)
        nc.scalar.dma_start(out=wo_sb, in_=wo_d)
        nc.vector.memset(KH[d:d + 1, :], -1.0)
        nc.vector.memset(v_sb[:, :, d:d + 1], 1.0)

        def k_proj(nb):
            nbs = ds(nb * 512, 512)
            kth_t = kstream.tile([P, KC, 512], f16, tag="kth", name="kth")
            nc.sync.dma_start(out=kth_t,
                              in_=kth_d[:, nbs].rearrange("(c p) x -> p c x", p=P))
            ktl_t = kstream.tile([P, KC, 512], f16, tag="ktl", name="ktl")
            nc.sync.dma_start(out=ktl_t,
                              in_=ktl_d[:, nbs].rearrange("(c p) x -> p c x", p=P))
            ps_k = pk_pool.tile([d, 512], f32)
            terms = [(wkh_sb, kth_t), (wkh_sb, ktl_t), (wkl_sb, kth_t)]
            for i, (w, x) in enumerate(terms):
                for kc in range(KC):
                    nc.tensor.matmul(ps_k, lhsT=w[:, kc, :], rhs=x[:, kc, :],
                                     start=(i == 0 and kc == 0),
                                     stop=(i == 2 and kc == KC - 1))
            nc.scalar.copy(KH[0:d, nbs], ps_k)
            nc.vector.tensor_sub(KX[0:d, nbs], ps_k, KH[0:d, nbs])
            nc.sync.dma_start(out=KX[d:2 * d, nbs], in_=KH[0:d, nbs])

        # SP-queue DMA order is the P1 critical path: qt0 (stats needs
        # chunk-0 q), then all of K, then qt1/qt2; VT rides the ACT queue.
        q_dma(0)
        ps_q0 = pq_pool.tile([d, 512], f32)
        q_proj_mms(0, ps_q0, 0, 12)
        q_evict(0, ps_q0)
        k_proj(0)
        k_proj(1)
        for j in range(NTC):
            stats_item(0, j * NTC + 0, st0_pool, "st0")
        for nb in range(2, NB):
            k_proj(nb)
            if nb % 2 == 1:
                i = nb // 2
                for j in range(NTC):
                    stats_item(0, j * NTC + i, st0_pool, "st0")
        q_dma(1)
        q_dma(2)
        for nb in (1, 2):
            ps_qx = pq_pool.tile([d, 512], f32, name="ps_qx")
            q_proj_mms(nb, ps_qx, 0, 12)
            q_evict(nb, ps_qx)
        # v DMAs issued here (SP queue, behind the q/k streams); all 32
        # projections are injected into era 0, two per group, staying >= 2
        # groups ahead of their attn.V use
        vt_tiles = [v_dma(mt) for mt in range(NT)]

    # ---- eras: main + stats(c+1) + injected projections + Wo(c-1) ----
    with tc.tile_pool(name="big_ps_pool", bufs=3, space="PSUM") as big_pool, \
         tc.tile_pool(name="av_ps_pool", bufs=1, space="PSUM") as av_pool, \
         tc.tile_pool(name="aux_ps_pool", bufs=1, space="PSUM") as aux_pool, \
         tc.tile_pool(name="att_pool", bufs=6) as att_pool:

        def wo_tile(t, pool=None, tag="aux"):
            """Output-projection for n-tile t, scaled by 1/sumexp on eviction."""
            wops = (pool or aux_pool).tile([P, dim], f32, tag=tag, name="wops")
            nc.tensor.matmul(wops, lhsT=outT[:, ts(t, P)], rhs=wo_sb,
                             start=True, stop=True)
            o_sb = att_pool.tile([P, dim], f16, tag="o_sb", name="o_sb")
            nc.scalar.mul(o_sb, wops, rinv[:, t:t + 1])
            nc.sync.dma_start(out=out_d[ts(t, P), :], in_=o_sb)

        # stats item k of chunk c+1 runs at group _item_group[k]+2 of era c:
        # spread for even DVE load, finishing with >= 1.5 groups of slack
        _item_group = [0, 0, 1, 2, 3, 3, 4, 5, 6, 6, 7, 8, 9, 9, 10, 11]
        stats_sched = {}
        for _k, _g in enumerate(_item_group):
            stats_sched.setdefault(_g + 2, []).append(_k)

        def era(c):
            """Main pass for chunk c; stats(c+1), q_proj(c+2), Wo(c-1) woven in."""
            At = QH_ev if c % 2 == 0 else QH_od
            cs = ds(c * CH, CH)
            r65 = At[:, cs]   # [65, 512], row 64 = rowmax
            rx = QX[:, cs]    # [128, 512]: q_hi / q_lo
            av_ps = av_pool.tile([d + 1, 512], f32, tag="av")
            att_fifo = []  # (att_tile, g) awaiting attn.V, deferred 2 groups
            ps_q = [None]

            def emit_av(att_t, g):
                nc.tensor.matmul(av_ps, lhsT=v_sb[:, 2 * g, :], rhs=att_t[:, 0:512],
                                 start=(g == 0), stop=False)
                nc.tensor.matmul(av_ps, lhsT=v_sb[:, 2 * g + 1, :], rhs=att_t[:, 512:1024],
                                 start=False, stop=(g == GRP - 1))

            for g in range(GRP):
                if c + 1 < NCH:
                    for k in stats_sched.get(g, ()):
                        stats_item(c + 1, k, big_pool, "big")
                sc_ps = big_pool.tile([P, 1024], f32, tag="big", name="sc_ps")
                att_t = att_pool.tile([P, 1024], bf16, tag="att")
                nc.tensor.matmul(sc_ps[:, 0:512], lhsT=KH[:, ts(2 * g, P)], rhs=r65,
                                 start=True, stop=False)
                nc.tensor.matmul(sc_ps[:, 512:1024], lhsT=KH[:, ts(2 * g + 1, P)], rhs=r65,
                                 start=True, stop=False)
                nc.tensor.matmul(sc_ps[:, 0:512], lhsT=KX[:, ts(2 * g, P)], rhs=rx,
                                 start=False, stop=True)
                nc.tensor.matmul(sc_ps[:, 512:1024], lhsT=KX[:, ts(2 * g + 1, P)], rhs=rx,
                                 start=False, stop=True)
                nc.scalar.activation(att_t, sc_ps, AF.Exp)
                # injected work sits AFTER this group's score matmuls so an
                # aux-bank wait never head-of-line blocks the PE stream
                if 1 <= c and c + 2 < NCH:
                    if g == 0:
                        q_dma(c + 2)
                        ps_q[0] = aux_pool.tile([P, dim], f32, tag="aux", name="ps_q")
                    if g < 3:
                        q_proj_mms(c + 2, ps_q[0][0:d, :], 4 * g, 4 * (g + 1))
                    elif g == 3:
                        q_evict(c + 2, ps_q[0][0:d, :])
                if c == 0:
                    # v tiles 2g, 2g+1: attn.V (deferred 2 groups) first
                    # needs tile 2g-4 at group g, so this stays 2 ahead
                    v_proj(2 * g, vt_tiles[2 * g], aux_pool, "aux")
                    v_proj(2 * g + 1, vt_tiles[2 * g + 1], aux_pool, "aux")
                # defer attn.V two groups so the exp it reads is long done
                att_fifo.append((att_t, g))
                if len(att_fifo) > 2:
                    emit_av(*att_fifo.pop(0))
                # previous chunk's output projection, one tile per 4 groups
                if c >= 1 and g % NTC == 2:
                    wo_tile((c - 1) * NTC + g // NTC)
            for item in att_fifo:
                emit_av(*item)
            # evict attn_u @ v and the sumexp row, then gather denominators
            nc.scalar.copy(outT[:, cs], av_ps[0:d, :])
            nc.scalar.copy(sumx[:, cs], av_ps[d:d + 1, :])
            for jj in range(NTC):
                nc.sync.dma_start(out=rsum[:, c * NTC + jj:c * NTC + jj + 1],
                                  in_=sumx[:, ds(c * CH + jj * P, P)])
            nc.vector.reciprocal(rinv[:, ds(c * NTC, NTC)], rsum[:, ds(c * NTC, NTC)])

        for c in range(NCH):
            era(c)

        # ---- tail: last chunk's output projection (alternating PSUM
        # rings so the four tiles pipeline instead of serializing) ----
        for t in range((NCH - 1) * NTC, NT):
            if t % 2 == 0:
                wo_tile(t)
            else:
                wo_tile(t, big_pool, "big")


def make_in_maps(Q, K, V, Wq, Wk, Wv, Wo):
    """Host-side sharding: transpose activations, slice weights per head."""
    scale = 1.0 / np.sqrt(Wq.shape[-1])

    def _hilo(x):
        hi = x.astype(np.float16)
        lo = (x - hi.astype(np.float32)).astype(np.float16)
        return np.ascontiguousarray(hi), np.ascontiguousarray(lo)

    QTH, QTL = _hilo(np.asarray(Q).T.astype(np.float32))
    KTH, KTL = _hilo(np.asarray(K).T.astype(np.float32))
    VT = np.ascontiguousarray(np.asarray(V).T.astype(np.float16))
    d = Wq.shape[-1]
    in_maps = []
    for h in range(Wq.shape[0]):
        wqh, wql = _hilo(Wq[h].astype(np.float32) * scale)
        wkh, wkl = _hilo(Wk[h].astype(np.float32))
        in_maps.append({
            "QTH": QTH, "QTL": QTL, "KTH": KTH, "KTL": KTL, "VT": VT,
            "wqh": wqh, "wql": wql, "wkh": wkh, "wkl": wkl,
            "wv": np.ascontiguousarray(Wv[h].astype(np.float16)),
            "wo": np.ascontiguousarray(Wo[h * d:(h + 1) * d, :].astype(np.float16)),
        })
    return in_maps


_CACHE = {}


def _build_and_compile(n=N, dim=DIM, d=D, num_cores=H, repeats=1):
    import concourse.bass as bass
    import concourse.mybir as mybir
    import concourse.tile as tile
    from concourse import bacc

    key = (n, dim, d, num_cores, repeats)
    if key in _CACHE:
        return _CACHE[key]
    nc = bacc.Bacc("TRN2", target_bir_lowering=False, debug=False,
                   num_devices=num_cores)
    f32 = mybir.dt.float32
    f16 = mybir.dt.float16
    ins = {}
    for name in ("QTH", "QTL", "KTH", "KTL"):
        ins[name] = nc.dram_tensor(name, [dim, n], f16, kind="ExternalInput").ap()
    ins["VT"] = nc.dram_tensor("VT", [dim, n], f16, kind="ExternalInput").ap()
    for name in ("wqh", "wql", "wkh", "wkl"):
        ins[name] = nc.dram_tensor(name, [dim, d], f16, kind="ExternalInput").ap()
    ins["wv"] = nc.dram_tensor("wv", [dim, d], f16, kind="ExternalInput").ap()
    ins["wo"] = nc.dram_tensor("wo", [d, dim], f16, kind="ExternalInput").ap()
    outs = {"out": nc.dram_tensor("out", [n, dim], f16, kind="ExternalOutput").ap()}
    with tile.TileContext(nc) as tc:
        for _rep in range(repeats):
            with ExitStack() as ctx:
                build_head_kernel(ctx, tc, outs, ins, n=n, dim=dim, d=d)
    nc.compile()
    _CACHE[key] = nc
    return nc


def run_on_hw(in_maps, trace=False, **kwargs):
    from concourse.bass_utils import run_bass_kernel_spmd

    nc = _build_and_compile(num_cores=len(in_maps))
    return run_bass_kernel_spmd(nc, in_maps, core_ids=list(range(len(in_maps))),
                                trace=trace, **kwargs)


def kernel(Q, K, V, Wq, Wk, Wv, Wo):
    in_maps = make_in_maps(np.asarray(Q), np.asarray(K), np.asarray(V),
                           np.asarray(Wq), np.asarray(Wk), np.asarray(Wv),
                           np.asarray(Wo))
    res = run_on_hw(in_maps)
    out = np.zeros((N, DIM), dtype=np.float64)
    for r in res.results:
        out += r["out"].astype(np.float64)
    return out.astype(np.float32)


if __name__ == "__main__":
    rng = np.random.default_rng(0)
    inputs = {
        "Q": rng.standard_normal((N, DIM), dtype=np.float32),
        "K": rng.standard_normal((N, DIM), dtype=np.float32),
        "V": rng.standard_normal((N, DIM), dtype=np.float32),
        "Wq": rng.random((H, DIM, D), dtype=np.float32),
        "Wk": rng.random((H, DIM, D), dtype=np.float32),
        "Wv": rng.random((H, DIM, D), dtype=np.float32),
        "Wo": rng.random((DIM, DIM), dtype=np.float32),
    }
    out = kernel(**inputs)
    print(out.shape, out.dtype, np.abs(out).max())


# revision 38
# speedup vs baseline: 1.2660x; 1.0162x over previous
"""Multi-head attention on 8 Trainium2 NeuronCores (head-parallel).

Problem: Q,K,V [4096,512] fp32; Wq/Wk/Wv [8,512,64]; Wo [512,512].
  out = concat_h(softmax(QWq_h (KWk_h)^T / sqrt(64)) VWv_h) @ Wo

Sharding: one head per core. Each core computes its head end-to-end plus
its slice of the output projection (out_h @ Wo[64h:64h+64, :]); the host
sums the 8 partial [4096,512] outputs.

Per-core pipeline (n = 4096 queries, m = 4096 keys, d = 64):
  P1 (head): K projection + q projections for chunks 0-2 + chunk-0
      stats, all chasing the input DMA stream. Only the work era 0
      actually needs lives here; the v projections and the remaining q
      chunks are injected into the era pipeline so the PE never idles on
      a serial projection tail.
  Eras (one per 512-query chunk c):
      stats pass (fp16 hi*hi, for chunk c+1): natural-layout scores ->
        per-row max (DVE reduce over PSUM); row maxes are DMA-scattered
        into row 64 of the fp16 q operand. The max error (~|s|*2^-11)
        cancels exactly: softmax is shift-invariant and the denominator
        is computed from the same shifted weights.
      main pass (fp16 hi/lo, 2 matmuls per m-tile): transposed scores.
        Pass 1 is hi*hi with a 65th contraction row (k side = -1, q side
        = rowmax) so PSUM holds qk^T - rowmax directly; pass 2 fuses both
        cross terms as [k_lo;k_hi] x [q_hi;q_lo] at K=128. Only the
        lo*lo term (~|s|*2^-22) is dropped: scores here reach |s| ~ 1e4
        (the all-positive projection weights give q and k a large shared
        sign pattern), so ~2^-17 relative accuracy is required -- which
        also rules out single-pass fp32r (~2^-13) for scores AND for the
        projections (hence fp16 hi/lo 3-term projections).
      attn.V (bf16): accumulate outT [65, 512] in PSUM over all 32
        m-tiles; row 64 (ones column of v) is the softmax denominator.
        bf16 (not fp16) because exp(s - rowmax) can reach e^+12 from the
        stats/rowmax fp16 rounding.
      injected work: q projection for chunk c+2 (fp16 hi/lo 3-term), its
        hi/lo evictions + relocations, the odd-tile copy, the second half
        of the v tiles (era 0), and the PREVIOUS chunk's output
        projection (Wo, fp16), one n-tile per 4 groups, scaled by 1/sum
        on eviction. These share one spare PSUM bank (tag "aux").
  Tail: last chunk's Wo tiles.
"""

from contextlib import ExitStack

import numpy as np

N = 4096
DIM = 512
H = 8
D = 64
P = 128
CH = 512  # query columns per era (chunk)


def build_head_kernel(ctx, tc, outs, ins, n=N, dim=DIM, d=D):
    import concourse.bass as bass
    import concourse.mybir as mybir
    from concourse.bass import ts, ds

    nc = tc.nc
    f32 = mybir.dt.float32
    f16 = mybir.dt.float16
    bf16 = mybir.dt.bfloat16
    AF = mybir.ActivationFunctionType
    X = mybir.AxisListType.X

    KC = dim // P      # projection contraction chunks (4)
    NT = n // P        # 128-row tiles of n (= m tiles) (32)
    NCH = n // CH      # eras (8)
    NTC = CH // P      # n-tiles per era (4)
    NB = n // 512      # projection column blocks (8)
    GRP = NT // 2      # main groups per era, 2 m-tiles each (16)
    assert CH == 512 and n % 1024 == 0

    qth_d, qtl_d = ins["QTH"], ins["QTL"]
    kth_d, ktl_d = ins["KTH"], ins["KTL"]
    vt_d = ins["VT"]
    wqh_d, wql_d = ins["wqh"], ins["wql"]
    wkh_d, wkl_d = ins["wkh"], ins["wkl"]
    wv_d, wo_d = ins["wv"], ins["wo"]
    out_d = outs["out"]

    singles = ctx.enter_context(tc.tile_pool(name="singles", bufs=1))

    QH_ev = singles.tile([d + 1, n], f16)   # rows 0-63 q_hi; row 64 rowmax
    QH_od = singles.tile([d + 1, n], f16)
    KH = singles.tile([d + 1, n], f16)      # rows 0-63 k_hi; row 64 = -1
    QX = singles.tile([P, n], f16)          # rows 0-63 q_hi, 64-127 q_lo
    KX = singles.tile([P, n], f16)          # rows 0-63 k_lo, 64-127 k_hi
    v_sb = singles.tile([P, NT, d + 1], bf16)  # v tiles + ones column
    outT = singles.tile([d, n], bf16)       # attn_u @ v
    sumx = singles.tile([1, n], f32)        # softmax denominators
    rsum = singles.tile([P, NT], f32)       # sumexp gathered per n-tile
    rinv = singles.tile([P, NT], f32)
    wqh_sb = singles.tile([P, KC, d], f16)
    wql_sb = singles.tile([P, KC, d], f16)
    wkh_sb = singles.tile([P, KC, d], f16)
    wkl_sb = singles.tile([P, KC, d], f16)
    wv_sb = singles.tile([P, KC, d], f16)
    wo_sb = singles.tile([d, dim], f16)

    # q/v input streams live across eras: DMAs are issued ahead (P1 or the
    # preceding era) and the projection compute is injected later.
    qstream = ctx.enter_context(tc.tile_pool(name="qstream", bufs=3))
    vstream = ctx.enter_context(tc.tile_pool(name="vstream", bufs=34))
    nmax_pool = ctx.enter_context(tc.tile_pool(name="nmax_pool", bufs=6))

    qt_tiles = {}

    def q_dma(nb):
        nbs = ds(nb * 512, 512)
        qth_t = qstream.tile([P, KC, 512], f16, tag="qth", name="qth")
        nc.sync.dma_start(out=qth_t,
                          in_=qth_d[:, nbs].rearrange("(c p) x -> p c x", p=P))
        qtl_t = qstream.tile([P, KC, 512], f16, tag="qtl", name="qtl")
        nc.sync.dma_start(out=qtl_t,
                          in_=qtl_d[:, nbs].rearrange("(c p) x -> p c x", p=P))
        qt_tiles[nb] = (qth_t, qtl_t)

    def v_dma(mt):
        vt_t = vstream.tile([P, KC, P], f16, tag="vt", name="vt")
        nc.sync.dma_start(out=vt_t,
                            in_=vt_d[:, ts(mt, P)].rearrange("(c p) x -> p c x", p=P))
        return vt_t

    def q_proj_mms(nb, ps_q, lo, hi):
        """Projection matmul slots [lo, hi) of the 12 (term, kc) pairs."""
        qth_t, qtl_t = qt_tiles[nb]
        terms = [(wqh_sb, qth_t), (wql_sb, qth_t), (wqh_sb, qtl_t)]
        for s in range(lo, hi):
            i, kc = divmod(s, KC)
            w, x = terms[i]
            nc.tensor.matmul(ps_q, lhsT=w[:, kc, :], rhs=x[:, kc, :],
                             start=(s == 0), stop=(s == 3 * KC - 1))

    def q_evict(nb, ps_q):
        nbs = ds(nb * 512, 512)
        nc.scalar.copy(QH_ev[0:d, nbs], ps_q)
        nc.scalar.copy(QX[0:d, nbs], ps_q)
        qlt = qstream.tile([d, 512], f16, tag="qlt", name="qlt")
        nc.vector.tensor_sub(qlt, ps_q, QX[0:d, nbs])
        nc.sync.dma_start(out=QX[d:2 * d, nbs], in_=qlt)
        # odd-tile copy of this q chunk (row 64 is scattered separately)
        nc.sync.dma_start(out=QH_od[0:d, nbs], in_=QH_ev[0:d, nbs])
        del qt_tiles[nb]

    def v_proj(mt, vt_t, pool, tag):
        ps_v = pool.tile([P, dim], f32, tag=tag, name="ps_v")
        for kc in range(KC):
            nc.tensor.matmul(ps_v[:, 0:d], lhsT=vt_t[:, kc, :], rhs=wv_sb[:, kc, :],
                             start=(kc == 0), stop=(kc == KC - 1))
        nc.vector.tensor_copy(v_sb[:, mt, 0:d], ps_v[:, 0:d])

    nmax_tiles = {}

    def stats_item(c, k, pool, tag):
        """Stats for chunk c, item k: n-tile j = k//4 vs key block i = k%4."""
        j, i = divmod(k, NTC)
        gt = c * NTC + j  # global n-tile
        if i == 0:
            nmax_tiles[j] = nmax_pool.tile([P, NTC], f32, tag="nmax",
                                           name="nmax")
        st_ps = pool.tile([P, 1024], f32, tag=tag, name="st_ps")
        nc.tensor.matmul(st_ps[:, 0:512], lhsT=QH_ev[0:d, ts(gt, P)],
                         rhs=KH[0:d, ds(i * 1024, 512)], start=True, stop=True)
        nc.tensor.matmul(st_ps[:, 512:1024], lhsT=QH_ev[0:d, ts(gt, P)],
                         rhs=KH[0:d, ds(i * 1024 + 512, 512)], start=True, stop=True)
        nc.vector.reduce_max(nmax_tiles[j][:, i:i + 1], st_ps, axis=X)
        if i == NTC - 1:
            cm = nmax_pool.tile([P, 1], f16, tag="cm", name="cm")
            nc.vector.reduce_max(cm, nmax_tiles[j], axis=X)
            At = QH_ev if c % 2 == 0 else QH_od
            # scatter per-row maxes into row 64: column n = c*CH + j*P + row
            nc.sync.dma_start(out=At[d:d + 1, ds(c * CH + j * P, P)], in_=cm)

    # ---- P1 head: K + chunk-0 q + chunk-0 stats + first-half v tiles ----
    with tc.tile_pool(name="kstream", bufs=4) as kstream, \
         tc.tile_pool(name="pq_ps", bufs=1, space="PSUM") as pq_pool, \
         tc.tile_pool(name="pk_ps", bufs=1, space="PSUM") as pk_pool, \
         tc.tile_pool(name="st0_ps", bufs=2, space="PSUM") as st0_pool:

        # weights on the ACT hwdge queue so kth0 is the first SP transfer
        def _load_w(w_sb, w_d):
            nc.scalar.dma_start(out=w_sb, in_=w_d.rearrange("(c p) e -> p c e", p=P))
        _load_w(wkh_sb, wkh_d)
        _load_w(wkl_sb, wkl_d)
        _load_w(wqh_sb, wqh_d)
        _load_w(wql_sb, wql_d)
        _load_w(wv_sb, wv_d)
        nc.scalar.dma_start(out=wo_sb, in_=wo_d)
        nc.vector.memset(KH[d:d + 1, :], -1.0)
        nc.vector.memset(v_sb[:, :, d:d + 1], 1.0)

        def k_proj(nb):
            nbs = ds(nb * 512, 512)
            kth_t = kstream.tile([P, KC, 512], f16, tag="kth", name="kth")
            nc.sync.dma_start(out=kth_t,
                              in_=kth_d[:, nbs].rearrange("(c p) x -> p c x", p=P))
            ktl_t = kstream.tile([P, KC, 512], f16, tag="ktl", name="ktl")
            nc.sync.dma_start(out=ktl_t,
                              in_=ktl_d[:, nbs].rearrange("(c p) x -> p c x", p=P))
            ps_k = pk_pool.tile([d, 512], f32)
            terms = [(wkh_sb, kth_t), (wkl_sb, kth_t), (wkh_sb, ktl_t)]
            for i, (w, x) in enumerate(terms):
                for kc in range(KC):
                    nc.tensor.matmul(ps_k, lhsT=w[:, kc, :], rhs=x[:, kc, :],
                                     start=(i == 0 and kc == 0),
                                     stop=(i == 2 and kc == KC - 1))
            nc.scalar.copy(KH[0:d, nbs], ps_k)
            nc.vector.tensor_sub(KX[0:d, nbs], ps_k, KH[0:d, nbs])
            nc.sync.dma_start(out=KX[d:2 * d, nbs], in_=KH[0:d, nbs])

        # SP-queue DMA order is the P1 critical path: qt0 (stats needs
        # chunk-0 q), then all of K, then qt1/qt2; VT rides the ACT queue.
        q_dma(0)
        ps_q0 = pq_pool.tile([d, 512], f32)
        q_proj_mms(0, ps_q0, 0, 12)
        q_evict(0, ps_q0)
        k_proj(0)
        k_proj(1)
        for j in range(NTC):
            stats_item(0, j * NTC + 0, st0_pool, "st0")
        for nb in range(2, NB):
            k_proj(nb)
            if nb % 2 == 1:
                i = nb // 2
                for j in range(NTC):
                    stats_item(0, j * NTC + i, st0_pool, "st0")
        q_dma(1)
        q_dma(2)
        for nb in (1, 2):
            ps_qx = pq_pool.tile([d, 512], f32, name="ps_qx")
            q_proj_mms(nb, ps_qx, 0, 12)
            q_evict(nb, ps_qx)
        # v DMAs issued here (SP queue, behind the q/k streams); all 32
        # projections are injected into era 0, two per group, staying >= 2
        # groups ahead of their attn.V use
        vt_tiles = [v_dma(mt) for mt in range(NT)]

    # ---- eras: main + stats(c+1) + injected projections + Wo(c-1) ----
    with tc.tile_pool(name="big_ps_pool", bufs=3, space="PSUM") as big_pool, \
         tc.tile_pool(name="av_ps_pool", bufs=1, space="PSUM") as av_pool, \
         tc.tile_pool(name="aux_ps_pool", bufs=1, space="PSUM") as aux_pool, \
         tc.tile_pool(name="att_pool", bufs=8) as att_pool:

        def wo_tile(t, pool=None, tag="aux"):
            """Output-projection for n-tile t, scaled by 1/sumexp on eviction."""
            wops = (pool or aux_pool).tile([P, dim], f32, tag=tag, name="wops")
            nc.tensor.matmul(wops, lhsT=outT[:, ts(t, P)], rhs=wo_sb,
                             start=True, stop=True)
            o_sb = att_pool.tile([P, dim], f16, tag="o_sb", name="o_sb")
            nc.scalar.mul(o_sb, wops, rinv[:, t:t + 1])
            nc.sync.dma_start(out=out_d[ts(t, P), :], in_=o_sb)

        # stats item k of chunk c+1 runs at group _item_group[k]+2 of era c:
        # spread for even DVE load, finishing with >= 1.5 groups of slack
        _item_group = [0, 0, 1, 2, 3, 3, 4, 5, 6, 6, 7, 8, 9, 9, 10, 11]
        stats_sched = {}
        for _k, _g in enumerate(_item_group):
            stats_sched.setdefault(_g + 2, []).append(_k)

        def era(c):
            """Main pass for chunk c; stats(c+1), q_proj(c+2), Wo(c-1) woven in."""
            At = QH_ev if c % 2 == 0 else QH_od
            cs = ds(c * CH, CH)
            r65 = At[:, cs]   # [65, 512], row 64 = rowmax
            rx = QX[:, cs]    # [128, 512]: q_hi / q_lo
            av_ps = av_pool.tile([d + 1, 512], f32, tag="av")
            att_fifo = []  # (att_tile, g) awaiting attn.V, deferred 2 groups
            ps_q = [None]

            def emit_av(att_t, g):
                nc.tensor.matmul(av_ps, lhsT=v_sb[:, 2 * g, :], rhs=att_t[:, 0:512],
                                 start=(g == 0), stop=False)
                nc.tensor.matmul(av_ps, lhsT=v_sb[:, 2 * g + 1, :], rhs=att_t[:, 512:1024],
                                 start=False, stop=(g == GRP - 1))

            for g in range(GRP):
                sc_ps = big_pool.tile([P, 1024], f32, tag="big", name="sc_ps")
                att_t = att_pool.tile([P, 1024], bf16, tag="att")
                nc.tensor.matmul(sc_ps[:, 0:512], lhsT=KH[:, ts(2 * g, P)], rhs=r65,
                                 start=True, stop=False)
                nc.tensor.matmul(sc_ps[:, 512:1024], lhsT=KH[:, ts(2 * g + 1, P)], rhs=r65,
                                 start=True, stop=False)
                nc.tensor.matmul(sc_ps[:, 0:512], lhsT=KX[:, ts(2 * g, P)], rhs=rx,
                                 start=False, stop=True)
                nc.tensor.matmul(sc_ps[:, 512:1024], lhsT=KX[:, ts(2 * g + 1, P)], rhs=rx,
                                 start=False, stop=True)
                nc.scalar.activation(att_t, sc_ps, AF.Exp)
                if c + 1 < NCH:
                    for k in stats_sched.get(g, ()):
                        stats_item(c + 1, k, big_pool, "big")
                # injected work sits AFTER this group's score matmuls so an
                # aux-bank wait never head-of-line blocks the PE stream
                if 1 <= c and c + 2 < NCH:
                    if g == 0:
                        q_dma(c + 2)
                        ps_q[0] = aux_pool.tile([P, dim], f32, tag="aux", name="ps_q")
                    if g < 3:
                        q_proj_mms(c + 2, ps_q[0][0:d, :], 4 * g, 4 * (g + 1))
                    elif g == 3:
                        q_evict(c + 2, ps_q[0][0:d, :])
                if c == 0:
                    # v tiles 2g, 2g+1: attn.V (deferred 2 groups) first
                    # needs tile 2g-4 at group g, so this stays 2 ahead
                    v_proj(2 * g, vt_tiles[2 * g], aux_pool, "aux")
                    v_proj(2 * g + 1, vt_tiles[2 * g + 1], aux_pool, "aux")
                # defer attn.V two groups so the exp it reads is long done
                att_fifo.append((att_t, g))
                if len(att_fifo) > 4:
                    emit_av(*att_fifo.pop(0))
                # previous chunk's output projection, one tile per 4 groups
                if c >= 1 and g % NTC == 3:
                    wo_tile((c - 1) * NTC + g // NTC)
            for item in att_fifo:
                emit_av(*item)
            # evict attn_u @ v and the sumexp row, then gather denominators
            nc.vector.tensor_copy(outT[:, cs], av_ps[0:d, :])
            nc.vector.tensor_copy(sumx[:, cs], av_ps[d:d + 1, :])
            for jj in range(NTC):
                nc.sync.dma_start(out=rsum[:, c * NTC + jj:c * NTC + jj + 1],
                                  in_=sumx[:, ds(c * CH + jj * P, P)])
            nc.vector.reciprocal(rinv[:, ds(c * NTC, NTC)], rsum[:, ds(c * NTC, NTC)])

        for c in range(NCH):
            era(c)

        # ---- tail: last chunk's output projection (alternating PSUM
        # rings so the four tiles pipeline instead of serializing) ----
        for t in range((NCH - 1) * NTC, NT):
            if t % 2 == 0:
                wo_tile(t)
            else:
                wo_tile(t, big_pool, "big")


def make_in_maps(Q, K, V, Wq, Wk, Wv, Wo):
    """Host-side sharding: transpose activations, slice weights per head."""
    scale = 1.0 / np.sqrt(Wq.shape[-1])

    def _hilo(x):
        hi = x.astype(np.float16)
        lo = (x - hi.astype(np.float32)).astype(np.float16)
        return np.ascontiguousarray(hi), np.ascontiguousarray(lo)

    QTH, QTL = _hilo(np.asarray(Q).T.astype(np.float32))
    KTH, KTL = _hilo(np.asarray(K).T.astype(np.float32))
    VT = np.ascontiguousarray(np.asarray(V).T.astype(np.float16))
    d = Wq.shape[-1]
    in_maps = []
    for h in range(Wq.shape[0]):
        wqh, wql = _hilo(Wq[h].astype(np.float32) * scale)
        wkh, wkl = _hilo(Wk[h].astype(np.float32))
        in_maps.append({
            "QTH": QTH, "QTL": QTL, "KTH": KTH, "KTL": KTL, "VT": VT,
            "wqh": wqh, "wql": wql, "wkh": wkh, "wkl": wkl,
            "wv": np.ascontiguousarray(Wv[h].astype(np.float16)),
            "wo": np.ascontiguousarray(Wo[h * d:(h + 1) * d, :].astype(np.float16)),
        })
    return in_maps


_CACHE = {}


def _build_and_compile(n=N, dim=DIM, d=D, num_cores=H, repeats=1):
    import concourse.bass as bass
    import concourse.mybir as mybir
    import concourse.tile as tile
    from concourse import bacc

    key = (n, dim, d, num_cores, repeats)
    if key in _CACHE:
        return _CACHE[key]
    nc = bacc.Bacc("TRN2", target_bir_lowering=False, debug=False,
                   num_devices=num_cores)
    f32 = mybir.dt.float32
    f16 = mybir.dt.float16
    ins = {}
    for name in ("QTH", "QTL", "KTH", "KTL"):
        ins[name] = nc.dram_tensor(name, [dim, n], f16, kind="ExternalInput").ap()
    ins["VT"] = nc.dram_tensor("VT", [dim, n], f16, kind="ExternalInput").ap()
    for name in ("wqh", "wql", "wkh", "wkl"):
        ins[name] = nc.dram_tensor(name, [dim, d], f16, kind="ExternalInput").ap()
    ins["wv"] = nc.dram_tensor("wv", [dim, d], f16, kind="ExternalInput").ap()
    ins["wo"] = nc.dram_tensor("wo", [d, dim], f16, kind="ExternalInput").ap()
    outs = {"out": nc.dram_tensor("out", [n, dim], f16, kind="ExternalOutput").ap()}
    with tile.TileContext(nc) as tc:
        for _rep in range(repeats):
            with ExitStack() as ctx:
                build_head_kernel(ctx, tc, outs, ins, n=n, dim=dim, d=d)
    nc.compile()
    _CACHE[key] = nc
    return nc


def run_on_hw(in_maps, trace=False, **kwargs):
    from concourse.bass_utils import run_bass_kernel_spmd

    nc = _build_and_compile(num_cores=len(in_maps))
    return run_bass_kernel_spmd(nc, in_maps, core_ids=list(range(len(in_maps))),
                                trace=trace, **kwargs)


def kernel(Q, K, V, Wq, Wk, Wv, Wo):
    in_maps = make_in_maps(np.asarray(Q), np.asarray(K), np.asarray(V),
                           np.asarray(Wq), np.asarray(Wk), np.asarray(Wv),
                           np.asarray(Wo))
    res = run_on_hw(in_maps)
    out = np.zeros((N, DIM), dtype=np.float64)
    for r in res.results:
        out += r["out"].astype(np.float64)
    return out.astype(np.float32)


if __name__ == "__main__":
    rng = np.random.default_rng(0)
    inputs = {
        "Q": rng.standard_normal((N, DIM), dtype=np.float32),
        "K": rng.standard_normal((N, DIM), dtype=np.float32),
        "V": rng.standard_normal((N, DIM), dtype=np.float32),
        "Wq": rng.random((H, DIM, D), dtype=np.float32),
        "Wk": rng.random((H, DIM, D), dtype=np.float32),
        "Wv": rng.random((H, DIM, D), dtype=np.float32),
        "Wo": rng.random((DIM, DIM), dtype=np.float32),
    }
    out = kernel(**inputs)
    print(out.shape, out.dtype, np.abs(out).max())


# revision 41
# speedup vs baseline: 1.2741x; 1.0064x over previous
"""Multi-head attention on 8 Trainium2 NeuronCores (head-parallel).

Problem: Q,K,V [4096,512] fp32; Wq/Wk/Wv [8,512,64]; Wo [512,512].
  out = concat_h(softmax(QWq_h (KWk_h)^T / sqrt(64)) VWv_h) @ Wo

Sharding: one head per core. Each core computes its head end-to-end plus
its slice of the output projection (out_h @ Wo[64h:64h+64, :]); the host
sums the 8 partial [4096,512] outputs.

Per-core pipeline (n = 4096 queries, m = 4096 keys, d = 64):
  P1 (head): K projection + q projections for chunks 0-2 + chunk-0
      stats, all chasing the input DMA stream. Only the work era 0
      actually needs lives here; the v projections and the remaining q
      chunks are injected into the era pipeline so the PE never idles on
      a serial projection tail.
  Eras (one per 512-query chunk c):
      stats pass (fp16 hi*hi, for chunk c+1): natural-layout scores ->
        per-row max (DVE reduce over PSUM); row maxes are DMA-scattered
        into row 64 of the fp16 q operand. The max error (~|s|*2^-11)
        cancels exactly: softmax is shift-invariant and the denominator
        is computed from the same shifted weights.
      main pass (fp16 hi/lo, 2 matmuls per m-tile): transposed scores.
        Pass 1 is hi*hi with a 65th contraction row (k side = -1, q side
        = rowmax) so PSUM holds qk^T - rowmax directly; pass 2 fuses both
        cross terms as [k_lo;k_hi] x [q_hi;q_lo] at K=128. Only the
        lo*lo term (~|s|*2^-22) is dropped: scores here reach |s| ~ 1e4
        (the all-positive projection weights give q and k a large shared
        sign pattern), so ~2^-17 relative accuracy is required -- which
        also rules out single-pass fp32r (~2^-13) for scores AND for the
        projections (hence fp16 hi/lo 3-term projections).
      attn.V (bf16): accumulate outT [65, 512] in PSUM over all 32
        m-tiles; row 64 (ones column of v) is the softmax denominator.
        bf16 (not fp16) because exp(s - rowmax) can reach e^+12 from the
        stats/rowmax fp16 rounding.
      injected work: q projection for chunk c+2 (fp16 hi/lo 3-term), its
        hi/lo evictions + relocations, the odd-tile copy, the second half
        of the v tiles (era 0), and the PREVIOUS chunk's output
        projection (Wo, fp16), one n-tile per 4 groups, scaled by 1/sum
        on eviction. These share one spare PSUM bank (tag "aux").
  Tail: last chunk's Wo tiles.
"""

from contextlib import ExitStack

import numpy as np

N = 4096
DIM = 512
H = 8
D = 64
P = 128
CH = 512  # query columns per era (chunk)


def build_head_kernel(ctx, tc, outs, ins, n=N, dim=DIM, d=D):
    import concourse.bass as bass
    import concourse.mybir as mybir
    from concourse.bass import ts, ds

    nc = tc.nc
    f32 = mybir.dt.float32
    f16 = mybir.dt.float16
    bf16 = mybir.dt.bfloat16
    AF = mybir.ActivationFunctionType
    X = mybir.AxisListType.X

    KC = dim // P      # projection contraction chunks (4)
    NT = n // P        # 128-row tiles of n (= m tiles) (32)
    NCH = n // CH      # eras (8)
    NTC = CH // P      # n-tiles per era (4)
    NB = n // 512      # projection column blocks (8)
    GRP = NT // 2      # main groups per era, 2 m-tiles each (16)
    assert CH == 512 and n % 1024 == 0

    qth_d, qtl_d = ins["QTH"], ins["QTL"]
    kth_d, ktl_d = ins["KTH"], ins["KTL"]
    vt_d = ins["VT"]
    wqh_d, wql_d = ins["wqh"], ins["wql"]
    wkh_d, wkl_d = ins["wkh"], ins["wkl"]
    wv_d, wo_d = ins["wv"], ins["wo"]
    out_d = outs["out"]

    singles = ctx.enter_context(tc.tile_pool(name="singles", bufs=1))

    QH_ev = singles.tile([d + 1, n], f16)   # rows 0-63 q_hi; row 64 rowmax
    QH_od = singles.tile([d + 1, n], f16)
    KH = singles.tile([d + 1, n], f16)      # rows 0-63 k_hi; row 64 = -1
    QX = singles.tile([P, n], f16)          # rows 0-63 q_hi, 64-127 q_lo
    KX = singles.tile([P, n], f16)          # rows 0-63 k_lo, 64-127 k_hi
    v_sb = singles.tile([P, NT, d + 1], bf16)  # v tiles + ones column
    outT = singles.tile([d, n], bf16)       # attn_u @ v
    sumx = singles.tile([1, n], f32)        # softmax denominators
    rsum = singles.tile([P, NT], f32)       # sumexp gathered per n-tile
    rinv = singles.tile([P, NT], f32)
    wqh_sb = singles.tile([P, KC, d], f16)
    wql_sb = singles.tile([P, KC, d], f16)
    wkh_sb = singles.tile([P, KC, d], f16)
    wkl_sb = singles.tile([P, KC, d], f16)
    wv_sb = singles.tile([P, KC, d], f16)
    wo_sb = singles.tile([d, dim], f16)

    # q/v input streams live across eras: DMAs are issued ahead (P1 or the
    # preceding era) and the projection compute is injected later.
    qstream = ctx.enter_context(tc.tile_pool(name="qstream", bufs=3))
    vstream = ctx.enter_context(tc.tile_pool(name="vstream", bufs=34))
    nmax_pool = ctx.enter_context(tc.tile_pool(name="nmax_pool", bufs=6))

    qt_tiles = {}

    def q_dma(nb):
        nbs = ds(nb * 512, 512)
        qth_t = qstream.tile([P, KC, 512], f16, tag="qth", name="qth")
        nc.sync.dma_start(out=qth_t,
                          in_=qth_d[:, nbs].rearrange("(c p) x -> p c x", p=P))
        qtl_t = qstream.tile([P, KC, 512], f16, tag="qtl", name="qtl")
        nc.sync.dma_start(out=qtl_t,
                          in_=qtl_d[:, nbs].rearrange("(c p) x -> p c x", p=P))
        qt_tiles[nb] = (qth_t, qtl_t)

    def v_dma(mt):
        vt_t = vstream.tile([P, KC, P], f16, tag="vt", name="vt")
        nc.sync.dma_start(out=vt_t,
                            in_=vt_d[:, ts(mt, P)].rearrange("(c p) x -> p c x", p=P))
        return vt_t

    def q_proj_mms(nb, ps_q, lo, hi):
        """Projection matmul slots [lo, hi) of the 12 (term, kc) pairs."""
        qth_t, qtl_t = qt_tiles[nb]
        terms = [(wqh_sb, qth_t), (wql_sb, qth_t), (wqh_sb, qtl_t)]
        for s in range(lo, hi):
            i, kc = divmod(s, KC)
            w, x = terms[i]
            nc.tensor.matmul(ps_q, lhsT=w[:, kc, :], rhs=x[:, kc, :],
                             start=(s == 0), stop=(s == 3 * KC - 1))

    def q_evict(nb, ps_q):
        nbs = ds(nb * 512, 512)
        nc.scalar.copy(QH_ev[0:d, nbs], ps_q)
        nc.scalar.copy(QX[0:d, nbs], ps_q)
        qlt = qstream.tile([d, 512], f16, tag="qlt", name="qlt")
        nc.vector.tensor_sub(qlt, ps_q, QX[0:d, nbs])
        nc.sync.dma_start(out=QX[d:2 * d, nbs], in_=qlt)
        # odd-tile copy of this q chunk (row 64 is scattered separately)
        nc.sync.dma_start(out=QH_od[0:d, nbs], in_=QH_ev[0:d, nbs])
        del qt_tiles[nb]

    def v_proj(mt, vt_t, pool, tag):
        ps_v = pool.tile([P, dim], f32, tag=tag, name="ps_v")
        for kc in range(KC):
            nc.tensor.matmul(ps_v[:, 0:d], lhsT=vt_t[:, kc, :], rhs=wv_sb[:, kc, :],
                             start=(kc == 0), stop=(kc == KC - 1))
        nc.vector.tensor_copy(v_sb[:, mt, 0:d], ps_v[:, 0:d])

    nmax_tiles = {}

    def stats_item(c, k, pool, tag):
        """Stats for chunk c, item k: n-tile j = k//4 vs key block i = k%4."""
        j, i = divmod(k, NTC)
        gt = c * NTC + j  # global n-tile
        if i == 0:
            nmax_tiles[j] = nmax_pool.tile([P, NTC], f32, tag="nmax",
                                           name="nmax")
        st_ps = pool.tile([P, 1024], f32, tag=tag, name="st_ps")
        nc.tensor.matmul(st_ps[:, 0:512], lhsT=QH_ev[0:d, ts(gt, P)],
                         rhs=KH[0:d, ds(i * 1024, 512)], start=True, stop=True)
        nc.tensor.matmul(st_ps[:, 512:1024], lhsT=QH_ev[0:d, ts(gt, P)],
                         rhs=KH[0:d, ds(i * 1024 + 512, 512)], start=True, stop=True)
        nc.vector.reduce_max(nmax_tiles[j][:, i:i + 1], st_ps, axis=X)
        if i == NTC - 1:
            cm = nmax_pool.tile([P, 1], f16, tag="cm", name="cm")
            nc.vector.reduce_max(cm, nmax_tiles[j], axis=X)
            At = QH_ev if c % 2 == 0 else QH_od
            # scatter per-row maxes into row 64: column n = c*CH + j*P + row
            nc.sync.dma_start(out=At[d:d + 1, ds(c * CH + j * P, P)], in_=cm)

    # ---- P1 head: K + chunk-0 q + chunk-0 stats + first-half v tiles ----
    with tc.tile_pool(name="kstream", bufs=4) as kstream, \
         tc.tile_pool(name="pq_ps", bufs=1, space="PSUM") as pq_pool, \
         tc.tile_pool(name="pk_ps", bufs=1, space="PSUM") as pk_pool, \
         tc.tile_pool(name="st0_ps", bufs=2, space="PSUM") as st0_pool:

        # weights on the ACT hwdge queue so kth0 is the first SP transfer
        def _load_w(w_sb, w_d):
            nc.scalar.dma_start(out=w_sb, in_=w_d.rearrange("(c p) e -> p c e", p=P))
        _load_w(wkh_sb, wkh_d)
        _load_w(wkl_sb, wkl_d)
        _load_w(wqh_sb, wqh_d)
        _load_w(wql_sb, wql_d)
        _load_w(wv_sb, wv_d)
        nc.scalar.dma_start(out=wo_sb, in_=wo_d)
        nc.vector.memset(KH[d:d + 1, :], -1.0)
        nc.vector.memset(v_sb[:, :, d:d + 1], 1.0)

        def k_proj(nb):
            nbs = ds(nb * 512, 512)
            kth_t = kstream.tile([P, KC, 512], f16, tag="kth", name="kth")
            nc.sync.dma_start(out=kth_t,
                              in_=kth_d[:, nbs].rearrange("(c p) x -> p c x", p=P))
            ktl_t = kstream.tile([P, KC, 512], f16, tag="ktl", name="ktl")
            nc.sync.dma_start(out=ktl_t,
                              in_=ktl_d[:, nbs].rearrange("(c p) x -> p c x", p=P))
            ps_k = pk_pool.tile([d, 512], f32)
            terms = [(wkh_sb, kth_t), (wkl_sb, kth_t), (wkh_sb, ktl_t)]
            for i, (w, x) in enumerate(terms):
                for kc in range(KC):
                    nc.tensor.matmul(ps_k, lhsT=w[:, kc, :], rhs=x[:, kc, :],
                                     start=(i == 0 and kc == 0),
                                     stop=(i == 2 and kc == KC - 1))
            nc.scalar.copy(KH[0:d, nbs], ps_k)
            nc.vector.tensor_sub(KX[0:d, nbs], ps_k, KH[0:d, nbs])
            nc.sync.dma_start(out=KX[d:2 * d, nbs], in_=KH[0:d, nbs])

        # SP-queue DMA order is the P1 critical path: qt0 (stats needs
        # chunk-0 q), then all of K, then qt1/qt2; VT rides the ACT queue.
        q_dma(0)
        ps_q0 = pq_pool.tile([d, 512], f32)
        q_proj_mms(0, ps_q0, 0, 12)
        q_evict(0, ps_q0)
        k_proj(0)
        k_proj(1)
        for j in range(NTC):
            stats_item(0, j * NTC + 0, st0_pool, "st0")
        for nb in range(2, NB):
            k_proj(nb)
            if nb % 2 == 1:
                i = nb // 2
                for j in range(NTC):
                    stats_item(0, j * NTC + i, st0_pool, "st0")
        q_dma(1)
        q_dma(2)
        # q1 and q2 on separate PSUM rings (pq / the now-idle st0 ring) so
        # their matmul chains pipeline instead of serializing on one bank
        ps_q1 = pq_pool.tile([d, 512], f32, name="ps_q1")
        ps_q2 = st0_pool.tile([d, 512], f32, tag="st0", name="ps_q2")
        for s in range(0, 12, 4):
            q_proj_mms(1, ps_q1, s, s + 4)
            q_proj_mms(2, ps_q2, s, s + 4)
        q_evict(1, ps_q1)
        q_evict(2, ps_q2)
        # v DMAs issued here (SP queue, behind the q/k streams); all 32
        # projections are injected into era 0, two per group, staying >= 2
        # groups ahead of their attn.V use
        vt_tiles = [v_dma(mt) for mt in range(NT)]

    # ---- eras: main + stats(c+1) + injected projections + Wo(c-1) ----
    with tc.tile_pool(name="big_ps_pool", bufs=3, space="PSUM") as big_pool, \
         tc.tile_pool(name="av_ps_pool", bufs=1, space="PSUM") as av_pool, \
         tc.tile_pool(name="aux_ps_pool", bufs=1, space="PSUM") as aux_pool, \
         tc.tile_pool(name="att_pool", bufs=8) as att_pool:

        def wo_tile(t, pool=None, tag="aux"):
            """Output-projection for n-tile t, scaled by 1/sumexp on eviction."""
            wops = (pool or aux_pool).tile([P, dim], f32, tag=tag, name="wops")
            nc.tensor.matmul(wops, lhsT=outT[:, ts(t, P)], rhs=wo_sb,
                             start=True, stop=True)
            o_sb = att_pool.tile([P, dim], f16, tag="o_sb", name="o_sb")
            nc.scalar.mul(o_sb, wops, rinv[:, t:t + 1])
            nc.sync.dma_start(out=out_d[ts(t, P), :], in_=o_sb)

        # stats item k of chunk c+1 runs at group _item_group[k]+2 of era c:
        # spread for even DVE load, finishing with >= 1.5 groups of slack
        _item_group = [0, 0, 1, 2, 3, 3, 4, 5, 6, 6, 7, 8, 9, 9, 10, 11]
        stats_sched = {}
        for _k, _g in enumerate(_item_group):
            stats_sched.setdefault(_g + 2, []).append(_k)

        def era(c):
            """Main pass for chunk c; stats(c+1), q_proj(c+2), Wo(c-1) woven in."""
            At = QH_ev if c % 2 == 0 else QH_od
            cs = ds(c * CH, CH)
            r65 = At[:, cs]   # [65, 512], row 64 = rowmax
            rx = QX[:, cs]    # [128, 512]: q_hi / q_lo
            av_ps = av_pool.tile([d + 1, 512], f32, tag="av")
            att_fifo = []  # (att_tile, g) awaiting attn.V, deferred 2 groups
            ps_q = [None]

            def emit_av(att_t, g):
                nc.tensor.matmul(av_ps, lhsT=v_sb[:, 2 * g, :], rhs=att_t[:, 0:512],
                                 start=(g == 0), stop=False)
                nc.tensor.matmul(av_ps, lhsT=v_sb[:, 2 * g + 1, :], rhs=att_t[:, 512:1024],
                                 start=False, stop=(g == GRP - 1))

            for g in range(GRP):
                sc_ps = big_pool.tile([P, 1024], f32, tag="big", name="sc_ps")
                att_t = att_pool.tile([P, 1024], bf16, tag="att")
                nc.tensor.matmul(sc_ps[:, 0:512], lhsT=KH[:, ts(2 * g, P)], rhs=r65,
                                 start=True, stop=False)
                nc.tensor.matmul(sc_ps[:, 512:1024], lhsT=KH[:, ts(2 * g + 1, P)], rhs=r65,
                                 start=True, stop=False)
                nc.tensor.matmul(sc_ps[:, 0:512], lhsT=KX[:, ts(2 * g, P)], rhs=rx,
                                 start=False, stop=True)
                nc.tensor.matmul(sc_ps[:, 512:1024], lhsT=KX[:, ts(2 * g + 1, P)], rhs=rx,
                                 start=False, stop=True)
                nc.scalar.activation(att_t, sc_ps, AF.Exp)
                if c + 1 < NCH:
                    for k in stats_sched.get(g, ()):
                        stats_item(c + 1, k, big_pool, "big")
                # injected work sits AFTER this group's score matmuls so an
                # aux-bank wait never head-of-line blocks the PE stream
                if 1 <= c and c + 2 < NCH:
                    if g == 0:
                        q_dma(c + 2)
                        ps_q[0] = aux_pool.tile([P, dim], f32, tag="aux", name="ps_q")
                    if g < 3:
                        q_proj_mms(c + 2, ps_q[0][0:d, :], 4 * g, 4 * (g + 1))
                    elif g == 3:
                        q_evict(c + 2, ps_q[0][0:d, :])
                if c == 0:
                    # v tiles 2g, 2g+1: attn.V (deferred 2 groups) first
                    # needs tile 2g-4 at group g, so this stays 2 ahead
                    v_proj(2 * g, vt_tiles[2 * g], aux_pool, "aux")
                    v_proj(2 * g + 1, vt_tiles[2 * g + 1], aux_pool, "aux")
                # defer attn.V two groups so the exp it reads is long done
                att_fifo.append((att_t, g))
                if len(att_fifo) > 4:
                    emit_av(*att_fifo.pop(0))
                # previous chunk's output projection, one tile per 4 groups
                if c >= 1 and g % NTC == 3:
                    wo_tile((c - 1) * NTC + g // NTC)
            for item in att_fifo:
                emit_av(*item)
            # evict attn_u @ v and the sumexp row, then gather denominators
            nc.vector.tensor_copy(outT[:, cs], av_ps[0:d, :])
            nc.vector.tensor_copy(sumx[:, cs], av_ps[d:d + 1, :])
            for jj in range(NTC):
                nc.sync.dma_start(out=rsum[:, c * NTC + jj:c * NTC + jj + 1],
                                  in_=sumx[:, ds(c * CH + jj * P, P)])
            nc.vector.reciprocal(rinv[:, ds(c * NTC, NTC)], rsum[:, ds(c * NTC, NTC)])

        for c in range(NCH):
            era(c)

        # ---- tail: last chunk's output projection (alternating PSUM
        # rings so the four tiles pipeline instead of serializing) ----
        for t in range((NCH - 1) * NTC, NT):
            if t % 2 == 0:
                wo_tile(t)
            else:
                wo_tile(t, big_pool, "big")


def make_in_maps(Q, K, V, Wq, Wk, Wv, Wo):
    """Host-side sharding: transpose activations, slice weights per head."""
    scale = 1.0 / np.sqrt(Wq.shape[-1])

    def _hilo(x):
        hi = x.astype(np.float16)
        lo = (x - hi.astype(np.float32)).astype(np.float16)
        return np.ascontiguousarray(hi), np.ascontiguousarray(lo)

    QTH, QTL = _hilo(np.asarray(Q).T.astype(np.float32))
    KTH, KTL = _hilo(np.asarray(K).T.astype(np.float32))
    VT = np.ascontiguousarray(np.asarray(V).T.astype(np.float16))
    d = Wq.shape[-1]
    in_maps = []
    for h in range(Wq.shape[0]):
        wqh, wql = _hilo(Wq[h].astype(np.float32) * scale)
        wkh, wkl = _hilo(Wk[h].astype(np.float32))
        in_maps.append({
            "QTH": QTH, "QTL": QTL, "KTH": KTH, "KTL": KTL, "VT": VT,
            "wqh": wqh, "wql": wql, "wkh": wkh, "wkl": wkl,
            "wv": np.ascontiguousarray(Wv[h].astype(np.float16)),
            "wo": np.ascontiguousarray(Wo[h * d:(h + 1) * d, :].astype(np.float16)),
        })
    return in_maps


_CACHE = {}


def _build_and_compile(n=N, dim=DIM, d=D, num_cores=H, repeats=1):
    import concourse.bass as bass
    import concourse.mybir as mybir
    import concourse.tile as tile
    from concourse import bacc

    key = (n, dim, d, num_cores, repeats)
    if key in _CACHE:
        return _CACHE[key]
    nc = bacc.Bacc("TRN2", target_bir_lowering=False, debug=False,
                   num_devices=num_cores)
    f32 = mybir.dt.float32
    f16 = mybir.dt.float16
    ins = {}
    for name in ("QTH", "QTL", "KTH", "KTL"):
        ins[name] = nc.dram_tensor(name, [dim, n], f16, kind="ExternalInput").ap()
    ins["VT"] = nc.dram_tensor("VT", [dim, n], f16, kind="ExternalInput").ap()
    for name in ("wqh", "wql", "wkh", "wkl"):
        ins[name] = nc.dram_tensor(name, [dim, d], f16, kind="ExternalInput").ap()
    ins["wv"] = nc.dram_tensor("wv", [dim, d], f16, kind="ExternalInput").ap()
    ins["wo"] = nc.dram_tensor("wo", [d, dim], f16, kind="ExternalInput").ap()
    outs = {"out": nc.dram_tensor("out", [n, dim], f16, kind="ExternalOutput").ap()}
    with tile.TileContext(nc) as tc:
        for _rep in range(repeats):
            with ExitStack() as ctx:
                build_head_kernel(ctx, tc, outs, ins, n=n, dim=dim, d=d)
    nc.compile()
    _CACHE[key] = nc
    return nc


def run_on_hw(in_maps, trace=False, **kwargs):
    from concourse.bass_utils import run_bass_kernel_spmd

    nc = _build_and_compile(num_cores=len(in_maps))
    return run_bass_kernel_spmd(nc, in_maps, core_ids=list(range(len(in_maps))),
                                trace=trace, **kwargs)


def kernel(Q, K, V, Wq, Wk, Wv, Wo):
    in_maps = make_in_maps(np.asarray(Q), np.asarray(K), np.asarray(V),
                           np.asarray(Wq), np.asarray(Wk), np.asarray(Wv),
                           np.asarray(Wo))
    res = run_on_hw(in_maps)
    out = np.zeros((N, DIM), dtype=np.float64)
    for r in res.results:
        out += r["out"].astype(np.float64)
    return out.astype(np.float32)


if __name__ == "__main__":
    rng = np.random.default_rng(0)
    inputs = {
        "Q": rng.standard_normal((N, DIM), dtype=np.float32),
        "K": rng.standard_normal((N, DIM), dtype=np.float32),
        "V": rng.standard_normal((N, DIM), dtype=np.float32),
        "Wq": rng.random((H, DIM, D), dtype=np.float32),
        "Wk": rng.random((H, DIM, D), dtype=np.float32),
        "Wv": rng.random((H, DIM, D), dtype=np.float32),
        "Wo": rng.random((DIM, DIM), dtype=np.float32),
    }
    out = kernel(**inputs)
    print(out.shape, out.dtype, np.abs(out).max())


# revision 43
# speedup vs baseline: 1.2767x; 1.0020x over previous
"""Multi-head attention on 8 Trainium2 NeuronCores (head-parallel).

Problem: Q,K,V [4096,512] fp32; Wq/Wk/Wv [8,512,64]; Wo [512,512].
  out = concat_h(softmax(QWq_h (KWk_h)^T / sqrt(64)) VWv_h) @ Wo

Sharding: one head per core. Each core computes its head end-to-end plus
its slice of the output projection (out_h @ Wo[64h:64h+64, :]); the host
sums the 8 partial [4096,512] outputs.

Per-core pipeline (n = 4096 queries, m = 4096 keys, d = 64):
  P1 (head): K projection + q projections for chunks 0-2 + chunk-0
      stats, all chasing the input DMA stream. Only the work era 0
      actually needs lives here; the v projections and the remaining q
      chunks are injected into the era pipeline so the PE never idles on
      a serial projection tail.
  Eras (one per 512-query chunk c):
      stats pass (fp16 hi*hi, for chunk c+1): natural-layout scores ->
        per-row max (DVE reduce over PSUM); row maxes are DMA-scattered
        into row 64 of the fp16 q operand. The max error (~|s|*2^-11)
        cancels exactly: softmax is shift-invariant and the denominator
        is computed from the same shifted weights.
      main pass (fp16 hi/lo, 2 matmuls per m-tile): transposed scores.
        Pass 1 is hi*hi with a 65th contraction row (k side = -1, q side
        = rowmax) so PSUM holds qk^T - rowmax directly; pass 2 fuses both
        cross terms as [k_lo;k_hi] x [q_hi;q_lo] at K=128. Only the
        lo*lo term (~|s|*2^-22) is dropped: scores here reach |s| ~ 1e4
        (the all-positive projection weights give q and k a large shared
        sign pattern), so ~2^-17 relative accuracy is required -- which
        also rules out single-pass fp32r (~2^-13) for scores AND for the
        projections (hence fp16 hi/lo 3-term projections).
      attn.V (bf16): accumulate outT [65, 512] in PSUM over all 32
        m-tiles; row 64 (ones column of v) is the softmax denominator.
        bf16 (not fp16) because exp(s - rowmax) can reach e^+12 from the
        stats/rowmax fp16 rounding.
      injected work: q projection for chunk c+2 (fp16 hi/lo 3-term), its
        hi/lo evictions + relocations, the odd-tile copy, the second half
        of the v tiles (era 0), and the PREVIOUS chunk's output
        projection (Wo, fp16), one n-tile per 4 groups, scaled by 1/sum
        on eviction. These share one spare PSUM bank (tag "aux").
  Tail: last chunk's Wo tiles.
"""

from contextlib import ExitStack

import numpy as np

N = 4096
DIM = 512
H = 8
D = 64
P = 128
CH = 512  # query columns per era (chunk)


def build_head_kernel(ctx, tc, outs, ins, n=N, dim=DIM, d=D):
    import concourse.bass as bass
    import concourse.mybir as mybir
    from concourse.bass import ts, ds

    nc = tc.nc
    f32 = mybir.dt.float32
    f16 = mybir.dt.float16
    bf16 = mybir.dt.bfloat16
    AF = mybir.ActivationFunctionType
    X = mybir.AxisListType.X

    KC = dim // P      # projection contraction chunks (4)
    NT = n // P        # 128-row tiles of n (= m tiles) (32)
    NCH = n // CH      # eras (8)
    NTC = CH // P      # n-tiles per era (4)
    NB = n // 512      # projection column blocks (8)
    GRP = NT // 2      # main groups per era, 2 m-tiles each (16)
    assert CH == 512 and n % 1024 == 0

    qth_d, qtl_d = ins["QTH"], ins["QTL"]
    kth_d, ktl_d = ins["KTH"], ins["KTL"]
    vt_d = ins["VT"]
    wqh_d, wql_d = ins["wqh"], ins["wql"]
    wkh_d, wkl_d = ins["wkh"], ins["wkl"]
    wv_d, wo_d = ins["wv"], ins["wo"]
    out_d = outs["out"]

    singles = ctx.enter_context(tc.tile_pool(name="singles", bufs=1))

    QH_ev = singles.tile([d + 1, n], f16)   # rows 0-63 q_hi; row 64 rowmax
    QH_od = singles.tile([d + 1, n], f16)
    KH = singles.tile([d + 1, n], f16)      # rows 0-63 k_hi; row 64 = -1
    QX = singles.tile([P, n], f16)          # rows 0-63 q_hi, 64-127 q_lo
    KX = singles.tile([P, n], f16)          # rows 0-63 k_lo, 64-127 k_hi
    v_sb = singles.tile([P, NT, d + 1], bf16)  # v tiles + ones column
    outT = singles.tile([d, n], bf16)       # attn_u @ v
    sumx = singles.tile([1, n], f32)        # softmax denominators
    rsum = singles.tile([P, NT], f32)       # sumexp gathered per n-tile
    rinv = singles.tile([P, NT], f32)
    wqh_sb = singles.tile([P, KC, d], f16)
    wql_sb = singles.tile([P, KC, d], f16)
    wkh_sb = singles.tile([P, KC, d], f16)
    wkl_sb = singles.tile([P, KC, d], f16)
    wv_sb = singles.tile([P, KC, d], f16)
    wo_sb = singles.tile([d, dim], f16)

    # q/v input streams live across eras: DMAs are issued ahead (P1 or the
    # preceding era) and the projection compute is injected later.
    qstream = ctx.enter_context(tc.tile_pool(name="qstream", bufs=3))
    vstream = ctx.enter_context(tc.tile_pool(name="vstream", bufs=34))
    nmax_pool = ctx.enter_context(tc.tile_pool(name="nmax_pool", bufs=6))

    qt_tiles = {}

    def q_dma(nb):
        nbs = ds(nb * 512, 512)
        qth_t = qstream.tile([P, KC, 512], f16, tag="qth", name="qth")
        nc.sync.dma_start(out=qth_t,
                          in_=qth_d[:, nbs].rearrange("(c p) x -> p c x", p=P))
        qtl_t = qstream.tile([P, KC, 512], f16, tag="qtl", name="qtl")
        nc.sync.dma_start(out=qtl_t,
                          in_=qtl_d[:, nbs].rearrange("(c p) x -> p c x", p=P))
        qt_tiles[nb] = (qth_t, qtl_t)

    def v_dma(mt):
        vt_t = vstream.tile([P, KC, P], f16, tag="vt", name="vt")
        nc.sync.dma_start(out=vt_t,
                            in_=vt_d[:, ts(mt, P)].rearrange("(c p) x -> p c x", p=P))
        return vt_t

    def q_proj_mms(nb, ps_q, lo, hi):
        """Projection matmul slots [lo, hi) of the 12 (term, kc) pairs."""
        qth_t, qtl_t = qt_tiles[nb]
        terms = [(wqh_sb, qth_t), (wql_sb, qth_t), (wqh_sb, qtl_t)]
        for s in range(lo, hi):
            i, kc = divmod(s, KC)
            w, x = terms[i]
            nc.tensor.matmul(ps_q, lhsT=w[:, kc, :], rhs=x[:, kc, :],
                             start=(s == 0), stop=(s == 3 * KC - 1))

    def q_evict(nb, ps_q):
        nbs = ds(nb * 512, 512)
        nc.scalar.copy(QH_ev[0:d, nbs], ps_q)
        nc.scalar.copy(QX[0:d, nbs], ps_q)
        qlt = qstream.tile([d, 512], f16, tag="qlt", name="qlt")
        nc.vector.tensor_sub(qlt, ps_q, QX[0:d, nbs])
        nc.sync.dma_start(out=QX[d:2 * d, nbs], in_=qlt)
        # odd-tile copy of this q chunk (row 64 is scattered separately)
        nc.sync.dma_start(out=QH_od[0:d, nbs], in_=QH_ev[0:d, nbs])
        del qt_tiles[nb]

    def v_proj(mt, vt_t, pool, tag):
        ps_v = pool.tile([P, dim], f32, tag=tag, name="ps_v")
        for kc in range(KC):
            nc.tensor.matmul(ps_v[:, 0:d], lhsT=vt_t[:, kc, :], rhs=wv_sb[:, kc, :],
                             start=(kc == 0), stop=(kc == KC - 1))
        nc.vector.tensor_copy(v_sb[:, mt, 0:d], ps_v[:, 0:d])

    nmax_tiles = {}

    def stats_item(c, k, pool, tag):
        """Stats for chunk c, item k: n-tile j = k//4 vs key block i = k%4."""
        j, i = divmod(k, NTC)
        gt = c * NTC + j  # global n-tile
        if i == 0:
            nmax_tiles[j] = nmax_pool.tile([P, NTC], f32, tag="nmax",
                                           name="nmax")
        st_ps = pool.tile([P, 1024], f32, tag=tag, name="st_ps")
        nc.tensor.matmul(st_ps[:, 0:512], lhsT=QH_ev[0:d, ts(gt, P)],
                         rhs=KH[0:d, ds(i * 1024, 512)], start=True, stop=True)
        nc.tensor.matmul(st_ps[:, 512:1024], lhsT=QH_ev[0:d, ts(gt, P)],
                         rhs=KH[0:d, ds(i * 1024 + 512, 512)], start=True, stop=True)
        nc.vector.reduce_max(nmax_tiles[j][:, i:i + 1], st_ps, axis=X)
        if i == NTC - 1:
            cm = nmax_pool.tile([P, 1], f16, tag="cm", name="cm")
            nc.vector.reduce_max(cm, nmax_tiles[j], axis=X)
            At = QH_ev if c % 2 == 0 else QH_od
            # scatter per-row maxes into row 64: column n = c*CH + j*P + row
            nc.sync.dma_start(out=At[d:d + 1, ds(c * CH + j * P, P)], in_=cm)

    # ---- P1 head: K + chunk-0 q + chunk-0 stats + first-half v tiles ----
    with tc.tile_pool(name="kstream", bufs=4) as kstream, \
         tc.tile_pool(name="pq_ps", bufs=1, space="PSUM") as pq_pool, \
         tc.tile_pool(name="pk_ps", bufs=1, space="PSUM") as pk_pool, \
         tc.tile_pool(name="st0_ps", bufs=2, space="PSUM") as st0_pool:

        # weights on the ACT hwdge queue so kth0 is the first SP transfer
        def _load_w(w_sb, w_d):
            nc.scalar.dma_start(out=w_sb, in_=w_d.rearrange("(c p) e -> p c e", p=P))
        _load_w(wkh_sb, wkh_d)
        _load_w(wkl_sb, wkl_d)
        _load_w(wqh_sb, wqh_d)
        _load_w(wql_sb, wql_d)
        _load_w(wv_sb, wv_d)
        nc.scalar.dma_start(out=wo_sb, in_=wo_d)
        nc.vector.memset(KH[d:d + 1, :], -1.0)
        nc.vector.memset(v_sb[:, :, d:d + 1], 1.0)

        def k_proj(nb):
            nbs = ds(nb * 512, 512)
            kth_t = kstream.tile([P, KC, 512], f16, tag="kth", name="kth")
            nc.sync.dma_start(out=kth_t,
                              in_=kth_d[:, nbs].rearrange("(c p) x -> p c x", p=P))
            ktl_t = kstream.tile([P, KC, 512], f16, tag="ktl", name="ktl")
            nc.sync.dma_start(out=ktl_t,
                              in_=ktl_d[:, nbs].rearrange("(c p) x -> p c x", p=P))
            ps_k = pk_pool.tile([d, 512], f32)
            terms = [(wkh_sb, kth_t), (wkl_sb, kth_t), (wkh_sb, ktl_t)]
            for i, (w, x) in enumerate(terms):
                for kc in range(KC):
                    nc.tensor.matmul(ps_k, lhsT=w[:, kc, :], rhs=x[:, kc, :],
                                     start=(i == 0 and kc == 0),
                                     stop=(i == 2 and kc == KC - 1))
            nc.scalar.copy(KH[0:d, nbs], ps_k)
            nc.vector.tensor_sub(KX[0:d, nbs], ps_k, KH[0:d, nbs])
            nc.sync.dma_start(out=KX[d:2 * d, nbs], in_=KH[0:d, nbs])

        # SP-queue DMA order is the P1 critical path: qt0 (stats needs
        # chunk-0 q), then all of K, then qt1/qt2; VT rides the ACT queue.
        q_dma(0)
        ps_q0 = pq_pool.tile([d, 512], f32)
        q_proj_mms(0, ps_q0, 0, 12)
        q_evict(0, ps_q0)
        k_proj(0)
        k_proj(1)
        for j in range(NTC):
            stats_item(0, j * NTC + 0, st0_pool, "st0")
        for nb in range(2, NB):
            k_proj(nb)
            if nb % 2 == 1:
                i = nb // 2
                for j in range(NTC):
                    stats_item(0, j * NTC + i, st0_pool, "st0")
        q_dma(1)
        q_dma(2)
        # q1 and q2 on separate PSUM rings (pq / the now-idle st0 ring) so
        # their matmul chains pipeline instead of serializing on one bank
        ps_q1 = pq_pool.tile([d, 512], f32, name="ps_q1")
        ps_q2 = st0_pool.tile([d, 512], f32, tag="st0", name="ps_q2")
        for s in range(0, 12, 4):
            q_proj_mms(1, ps_q1, s, s + 4)
            q_proj_mms(2, ps_q2, s, s + 4)
        q_evict(1, ps_q1)
        q_evict(2, ps_q2)
        # v DMAs issued here (SP queue, behind the q/k streams); all 32
        # projections are injected into era 0, two per group, staying >= 2
        # groups ahead of their attn.V use
        vt_tiles = [v_dma(mt) for mt in range(NT)]

    # ---- eras: main + stats(c+1) + injected projections + Wo(c-1) ----
    with tc.tile_pool(name="big_ps_pool", bufs=3, space="PSUM") as big_pool, \
         tc.tile_pool(name="av_ps_pool", bufs=1, space="PSUM") as av_pool, \
         tc.tile_pool(name="aux_ps_pool", bufs=1, space="PSUM") as aux_pool, \
         tc.tile_pool(name="att_pool", bufs=8) as att_pool:

        def wo_tile(t, pool=None, tag="aux"):
            """Output-projection for n-tile t, scaled by 1/sumexp on eviction."""
            wops = (pool or aux_pool).tile([P, dim], f32, tag=tag, name="wops")
            nc.tensor.matmul(wops, lhsT=outT[:, ts(t, P)], rhs=wo_sb,
                             start=True, stop=True)
            o_sb = att_pool.tile([P, dim], f16, tag="o_sb", name="o_sb")
            nc.scalar.mul(o_sb, wops, rinv[:, t:t + 1])
            nc.sync.dma_start(out=out_d[ts(t, P), :], in_=o_sb)

        # stats item k of chunk c+1 runs at group _item_group[k]+2 of era c:
        # spread for even DVE load, finishing with >= 1.5 groups of slack
        _item_group = [0, 0, 1, 2, 3, 3, 4, 5, 6, 6, 7, 8, 9, 9, 10, 11]
        stats_sched = {}
        for _k, _g in enumerate(_item_group):
            stats_sched.setdefault(_g + 1, []).append(_k)

        def era(c):
            """Main pass for chunk c; stats(c+1), q_proj(c+2), Wo(c-1) woven in."""
            At = QH_ev if c % 2 == 0 else QH_od
            cs = ds(c * CH, CH)
            r65 = At[:, cs]   # [65, 512], row 64 = rowmax
            rx = QX[:, cs]    # [128, 512]: q_hi / q_lo
            av_ps = av_pool.tile([d + 1, 512], f32, tag="av")
            att_fifo = []  # (att_tile, g) awaiting attn.V, deferred 2 groups
            ps_q = [None]

            def emit_av(att_t, g):
                nc.tensor.matmul(av_ps, lhsT=v_sb[:, 2 * g, :], rhs=att_t[:, 0:512],
                                 start=(g == 0), stop=False)
                nc.tensor.matmul(av_ps, lhsT=v_sb[:, 2 * g + 1, :], rhs=att_t[:, 512:1024],
                                 start=False, stop=(g == GRP - 1))

            for g in range(GRP):
                sc_ps = big_pool.tile([P, 1024], f32, tag="big", name="sc_ps")
                att_t = att_pool.tile([P, 1024], bf16, tag="att")
                nc.tensor.matmul(sc_ps[:, 0:512], lhsT=KH[:, ts(2 * g, P)], rhs=r65,
                                 start=True, stop=False)
                nc.tensor.matmul(sc_ps[:, 512:1024], lhsT=KH[:, ts(2 * g + 1, P)], rhs=r65,
                                 start=True, stop=False)
                nc.tensor.matmul(sc_ps[:, 0:512], lhsT=KX[:, ts(2 * g, P)], rhs=rx,
                                 start=False, stop=True)
                nc.tensor.matmul(sc_ps[:, 512:1024], lhsT=KX[:, ts(2 * g + 1, P)], rhs=rx,
                                 start=False, stop=True)
                nc.scalar.activation(att_t, sc_ps, AF.Exp)
                if c + 1 < NCH:
                    for k in stats_sched.get(g, ()):
                        stats_item(c + 1, k, big_pool, "big")
                # injected work sits AFTER this group's score matmuls so an
                # aux-bank wait never head-of-line blocks the PE stream
                if 1 <= c and c + 2 < NCH:
                    if g == 0:
                        q_dma(c + 2)
                        ps_q[0] = aux_pool.tile([P, dim], f32, tag="aux", name="ps_q")
                    if g < 3:
                        q_proj_mms(c + 2, ps_q[0][0:d, :], 4 * g, 4 * (g + 1))
                    elif g == 3:
                        q_evict(c + 2, ps_q[0][0:d, :])
                if c == 0:
                    # v tiles 2g, 2g+1: attn.V (deferred 2 groups) first
                    # needs tile 2g-4 at group g, so this stays 2 ahead
                    v_proj(2 * g, vt_tiles[2 * g], aux_pool, "aux")
                    v_proj(2 * g + 1, vt_tiles[2 * g + 1], aux_pool, "aux")
                # defer attn.V two groups so the exp it reads is long done
                att_fifo.append((att_t, g))
                if len(att_fifo) > 4:
                    emit_av(*att_fifo.pop(0))
                # previous chunk's output projection, one tile per 4 groups
                if c >= 1 and g % NTC == 3:
                    wo_tile((c - 1) * NTC + g // NTC)
            for item in att_fifo:
                emit_av(*item)
            # evict attn_u @ v and the sumexp row, then gather denominators
            nc.vector.tensor_copy(outT[:, cs], av_ps[0:d, :])
            nc.vector.tensor_copy(sumx[:, cs], av_ps[d:d + 1, :])
            for jj in range(NTC):
                nc.sync.dma_start(out=rsum[:, c * NTC + jj:c * NTC + jj + 1],
                                  in_=sumx[:, ds(c * CH + jj * P, P)])
            nc.vector.reciprocal(rinv[:, ds(c * NTC, NTC)], rsum[:, ds(c * NTC, NTC)])

        for c in range(NCH):
            era(c)

        # ---- tail: last chunk's output projection (alternating PSUM
        # rings so the four tiles pipeline instead of serializing) ----
        for t in range((NCH - 1) * NTC, NT):
            if t % 2 == 0:
                wo_tile(t)
            else:
                wo_tile(t, big_pool, "big")


def make_in_maps(Q, K, V, Wq, Wk, Wv, Wo):
    """Host-side sharding: transpose activations, slice weights per head."""
    scale = 1.0 / np.sqrt(Wq.shape[-1])

    def _hilo(x):
        hi = x.astype(np.float16)
        lo = (x - hi.astype(np.float32)).astype(np.float16)
        return np.ascontiguousarray(hi), np.ascontiguousarray(lo)

    QTH, QTL = _hilo(np.asarray(Q).T.astype(np.float32))
    KTH, KTL = _hilo(np.asarray(K).T.astype(np.float32))
    VT = np.ascontiguousarray(np.asarray(V).T.astype(np.float16))
    d = Wq.shape[-1]
    in_maps = []
    for h in range(Wq.shape[0]):
        wqh, wql = _hilo(Wq[h].astype(np.float32) * scale)
        wkh, wkl = _hilo(Wk[h].astype(np.float32))
        in_maps.append({
            "QTH": QTH, "QTL": QTL, "KTH": KTH, "KTL": KTL, "VT": VT,
            "wqh": wqh, "wql": wql, "wkh": wkh, "wkl": wkl,
            "wv": np.ascontiguousarray(Wv[h].astype(np.float16)),
            "wo": np.ascontiguousarray(Wo[h * d:(h + 1) * d, :].astype(np.float16)),
        })
    return in_maps


_CACHE = {}


def _build_and_compile(n=N, dim=DIM, d=D, num_cores=H, repeats=1):
    import concourse.bass as bass
    import concourse.mybir as mybir
    import concourse.tile as tile
    from concourse import bacc

    key = (n, dim, d, num_cores, repeats)
    if key in _CACHE:
        return _CACHE[key]
    nc = bacc.Bacc("TRN2", target_bir_lowering=False, debug=False,
                   num_devices=num_cores)
    f32 = mybir.dt.float32
    f16 = mybir.dt.float16
    ins = {}
    for name in ("QTH", "QTL", "KTH", "KTL"):
        ins[name] = nc.dram_tensor(name, [dim, n], f16, kind="ExternalInput").ap()
    ins["VT"] = nc.dram_tensor("VT", [dim, n], f16, kind="ExternalInput").ap()
    for name in ("wqh", "wql", "wkh", "wkl"):
        ins[name] = nc.dram_tensor(name, [dim, d], f16, kind="ExternalInput").ap()
    ins["wv"] = nc.dram_tensor("wv", [dim, d], f16, kind="ExternalInput").ap()
    ins["wo"] = nc.dram_tensor("wo", [d, dim], f16, kind="ExternalInput").ap()
    outs = {"out": nc.dram_tensor("out", [n, dim], f16, kind="ExternalOutput").ap()}
    with tile.TileContext(nc) as tc:
        for _rep in range(repeats):
            with ExitStack() as ctx:
                build_head_kernel(ctx, tc, outs, ins, n=n, dim=dim, d=d)
    nc.compile()
    _CACHE[key] = nc
    return nc


def run_on_hw(in_maps, trace=False, **kwargs):
    from concourse.bass_utils import run_bass_kernel_spmd

    nc = _build_and_compile(num_cores=len(in_maps))
    return run_bass_kernel_spmd(nc, in_maps, core_ids=list(range(len(in_maps))),
                                trace=trace, **kwargs)


def kernel(Q, K, V, Wq, Wk, Wv, Wo):
    in_maps = make_in_maps(np.asarray(Q), np.asarray(K), np.asarray(V),
                           np.asarray(Wq), np.asarray(Wk), np.asarray(Wv),
                           np.asarray(Wo))
    res = run_on_hw(in_maps)
    out = np.zeros((N, DIM), dtype=np.float64)
    for r in res.results:
        out += r["out"].astype(np.float64)
    return out.astype(np.float32)


if __name__ == "__main__":
    rng = np.random.default_rng(0)
    inputs = {
        "Q": rng.standard_normal((N, DIM), dtype=np.float32),
        "K": rng.standard_normal((N, DIM), dtype=np.float32),
        "V": rng.standard_normal((N, DIM), dtype=np.float32),
        "Wq": rng.random((H, DIM, D), dtype=np.float32),
        "Wk": rng.random((H, DIM, D), dtype=np.float32),
        "Wv": rng.random((H, DIM, D), dtype=np.float32),
        "Wo": rng.random((DIM, DIM), dtype=np.float32),
    }
    out = kernel(**inputs)
    print(out.shape, out.dtype, np.abs(out).max())
